# revision 1
# baseline (speedup 1.0000x reference)
"""Trainium2 Bass kernel for nn_CapsuleModel2 (capsule routing head).

Strategy (data-parallel, one image per NeuronCore, 8 cores):

Host-side algebraic folding:
  The whole per-pixel chain  1x1conv(poses) -> per-capsule vote conv ->
  positional-encoding linear  collapses into a single effective matmul:
     tokens_grid[(n,v), s] = Weff @ feat + (r(s)*w_d + b_eff)
  where Weff = W16 @ w_vote[n] @ w_poses[n]  (host-precomputed, 128x1280)
  and the positional encoding is rank-1 in the *grid position only*:
  pe = [(y-x)/128, (x-y)/128] so pe @ w_pos[:,16:18].T = r(s) * (wy-wx).
  That grid-constant [128,4096] table ships from the host.

Device pipeline per core (one image):
  1. tokens_grid = WeffT.T @ feat (bf16 matmul, fp32 psum) + PEGRID   [128,4096]
     z_grid = w_acts @ feat + b_acts (activation logits)              [8,4096]
  2. ap_gather (GPSIMD ucode) pulls the I*P=4096 point columns:
     tok_all[(n,v), (i,p)], zg[(n), (i,p)]
  3. Routing 1 via block-diagonal matmuls, 128-pt chunks:
     LT[pt,(n,o)] = tok_chunk.T @ blockdiag(Q1/4)        E = exp(LT)
     vals[pt,(n,j)] = (sigmoid(z)+1e-6) * (tok_chunk.T @ blockdiag(Wv1) | 1)
     numer/denom accumulate per instance: psum[j(17), o(64)] += vals_n.T @ E_n
  4. Routing 2 per instance (tiny): transpose, normalize, Q2 attention,
     class sigmoid -> out[16,19] per image.
"""

import sys

for _p in ("/opt/trn_rl_repo",):
    if _p not in sys.path:
        sys.path.insert(0, _p)

import numpy as np
import ml_dtypes

import concourse.bacc as bacc
import concourse.tile as tile
from concourse import mybir
from concourse import bass_utils

AF = mybir.ActivationFunctionType
ALU = mybir.AluOpType
F32 = mybir.dt.float32
BF16 = mybir.dt.bfloat16
I16 = mybir.dt.int16
BF16_NP = ml_dtypes.bfloat16

B, I, P = 8, 16, 256
CIN = 1280
NCAPS, DCAP, DV = 8, 32, 16
HF = WF = 64
S = HF * WF              # 4096 grid positions
NPTS = I * P             # 4096 gathered points
NOUT1, NCLS = 64, 19
KT = CIN // 128          # 10 contraction tiles
HALF = S // 2
ZW = S + 8               # z grid padded with a -inf slot for masked points
NCH = 32                 # routing-1 chunks of 128 points

_CACHE = {}


def _build_nc(phases=3, repeat=1, dma_split=1):
    nc = bacc.Bacc("TRN2", target_bir_lowering=False, debug=False, num_devices=8)

    din = {}

    def dram_in(name, shape, dt):
        din[name] = nc.dram_tensor(name, list(shape), dt, kind="ExternalInput").ap()
        return din[name]

    feat = dram_in("feat", (CIN, S), BF16)
    pegrid = dram_in("pegrid", (128, S), F32)
    weffT = dram_in("weffT", (CIN, 128), BF16)
    waT = dram_in("waT", (CIN, 8), BF16)
    bacts = dram_in("bacts", (8, 1), F32)
    bq1 = dram_in("bq1", (128, 512), F32)
    bwv1 = dram_in("bwv1", (128, 136), F32)
    exp8rep = dram_in("exp8rep", (128, 136), F32)
    q2sT = dram_in("q2sT", (16, NCLS), F32)
    wact1rep = dram_in("wact1rep", (64, 16), F32)
    wact2rep = dram_in("wact2rep", (NCLS, 16), F32)
    nbact1rep = dram_in("nbact1rep", (64, 1), F32)
    nbact2rep = dram_in("nbact2rep", (NCLS, 1), F32)
    ident = dram_in("ident", (128, 128), F32)
    gidx = dram_in("gidx", (128, NPTS // 16), I16)
    aidx = dram_in("aidx", (128, NPTS // 64), I16)

    out_cls = nc.dram_tensor("out_cls", [I, NCLS], F32, kind="ExternalOutput").ap()

    with tile.TileContext(nc) as tc:
        with (
            tc.tile_pool(name="cons", bufs=1) as cons,
            tc.tile_pool(name="grid", bufs=1) as grid,
            tc.tile_pool(name="feats", bufs=3) as feats,
            tc.tile_pool(name="rsb", bufs=3) as rsb,
            tc.tile_pool(name="small", bufs=2) as small,
        ):
            # ---- constants to SBUF ----
            # spread large DMAs over several engines' DMA queues
            dma_engs = [nc.sync, nc.scalar, nc.gpsimd][:dma_split]
            pegrid_sb = cons.tile([128, S], F32)
            for jq in range(4):
                dma_engs[jq % len(dma_engs)].dma_start(
                    out=pegrid_sb[:, jq * 1024 : (jq + 1) * 1024],
                    in_=pegrid[:, jq * 1024 : (jq + 1) * 1024],
                )
            weffT_sb = cons.tile([128, KT, 128], BF16)
            nc.sync.dma_start(
                out=weffT_sb[:], in_=weffT.rearrange("(k p) m -> p k m", p=128)
            )
            waT_sb = cons.tile([128, KT, 8], BF16)
            nc.sync.dma_start(
                out=waT_sb[:], in_=waT.rearrange("(k p) m -> p k m", p=128)
            )
            bacts_sb = cons.tile([8, 1], F32)
            nc.sync.dma_start(out=bacts_sb[:], in_=bacts)
            bq1_sb = cons.tile([128, 512], F32)
            nc.sync.dma_start(out=bq1_sb[:], in_=bq1)
            bwv1_sb = cons.tile([128, 136], F32)
            nc.sync.dma_start(out=bwv1_sb[:], in_=bwv1)
            exp8rep_sb = cons.tile([128, 136], F32)
            nc.sync.dma_start(out=exp8rep_sb[:], in_=exp8rep)
            q2sT_sb = cons.tile([16, NCLS], F32)
            nc.sync.dma_start(out=q2sT_sb[:], in_=q2sT)
            wact1rep_sb = cons.tile([64, 16], F32)
            nc.sync.dma_start(out=wact1rep_sb[:], in_=wact1rep)
            wact2rep_sb = cons.tile([NCLS, 16], F32)
            nc.sync.dma_start(out=wact2rep_sb[:], in_=wact2rep)
            nbact1rep_sb = cons.tile([64, 1], F32)
            nc.sync.dma_start(out=nbact1rep_sb[:], in_=nbact1rep)
            nbact2rep_sb = cons.tile([NCLS, 1], F32)
            nc.sync.dma_start(out=nbact2rep_sb[:], in_=nbact2rep)
            ident_sb = cons.tile([128, 128], F32)
            nc.sync.dma_start(out=ident_sb[:], in_=ident)
            gidx_sb = cons.tile([128, NPTS // 16], I16)
            nc.sync.dma_start(out=gidx_sb[:], in_=gidx)
            aidx_sb = cons.tile([128, NPTS // 64], I16)
            nc.sync.dma_start(out=aidx_sb[:], in_=aidx)

            # ---- persistent grid tensors ----
            tokens_sb = grid.tile([128, S], F32)
            # z grid replicated at 32-partition strides so one ap_gather can
            # split the 4096 points across 4 GPSIMD cores.
            z_rep = grid.tile([128, ZW], F32)
            tok_all = grid.tile([128, NPTS], F32)
            zg2 = grid.tile([128, NPTS // 4], F32)
            outcls_sb = grid.tile([NCLS, I], F32)
            nc.vector.memset(z_rep[:, :], 0.0)
            nc.vector.memset(z_rep[0:8, S:ZW], -10000.0)

            for rep in range(repeat):
                # ---- phase G: grid matmuls ----
                with tc.tile_pool(name=f"pgrid{rep}", bufs=1, space="PSUM") as pg:
                    for h in range(2):
                        pms = [
                            pg.tile([128, 512], F32, tag=f"pm{nn}", name=f"pm{nn}_{h}_{rep}")
                            for nn in range(4)
                        ]
                        pas = [
                            pg.tile([8, 512], F32, tag=f"pa{nn}", name=f"pa{nn}_{h}_{rep}")
                            for nn in range(4)
                        ]
                        for k in range(KT):
                            ft = feats.tile([128, HALF], BF16, tag="feat")
                            dma_engs[(h * KT + k) % len(dma_engs)].dma_start(
                                out=ft[:],
                                in_=feat[k * 128 : (k + 1) * 128, h * HALF : (h + 1) * HALF],
                            )
                            for nn in range(4):
                                nc.tensor.matmul(
                                    pms[nn][:],
                                    lhsT=weffT_sb[:, k, :],
                                    rhs=ft[:, nn * 512 : (nn + 1) * 512],
                                    start=(k == 0),
                                    stop=(k == KT - 1),
                                )
                            for nn in range(4):
                                nc.tensor.matmul(
                                    pas[nn][:],
                                    lhsT=waT_sb[:, k, :],
                                    rhs=ft[:, nn * 512 : (nn + 1) * 512],
                                    start=(k == 0),
                                    stop=(k == KT - 1),
                                )
                        for nn in range(4):
                            off = h * HALF + nn * 512
                            nc.vector.tensor_add(
                                out=tokens_sb[:, off : off + 512],
                                in0=pms[nn][:],
                                in1=pegrid_sb[:, off : off + 512],
                            )
                            nc.scalar.activation(
                                out=z_rep[0:8, off : off + 512],
                                in_=pas[nn][:],
                                func=AF.Identity,
                                bias=bacts_sb[:],
                                scale=1.0,
                            )

                if phases < 3:
                    nc.vector.memset(outcls_sb[:, :], 0.0)

                # replicate z rows 0:8 to partition bases 32/64/96
                if phases >= 2:
                    for mq in range(1, 4):
                        nc.sync.dma_start(
                            out=z_rep[32 * mq : 32 * mq + 8, :], in_=z_rep[0:8, :]
                        )

                # ---- phase H: gathers (GPSIMD ucode) ----
                QIDX = NPTS // 4
                if phases >= 2:
                  nc.gpsimd.ap_gather(
                    zg2[:],
                    z_rep[:],
                    aidx_sb[:],
                    channels=128,
                    num_elems=ZW,
                    d=1,
                    num_idxs=QIDX,
                )
                if phases >= 2:
                  for q in range(4):
                    nc.gpsimd.ap_gather(
                        tok_all[:, q * QIDX : (q + 1) * QIDX],
                        tokens_sb[:],
                        gidx_sb[:, q * (QIDX // 16) : (q + 1) * (QIDX // 16)],
                        channels=128,
                        num_elems=S,
                        d=1,
                        num_idxs=QIDX,
                    )

                # ---- phase R1 + R2: routing ----
                # Software-pipelined: the numer matmuls for chunk c-1 are emitted
                # after chunk c's L/V/A matmuls so the PE FIFO never stalls on
                # chunk c's exp/sigmoid post-processing.
                with (
                    tc.tile_pool(name=f"pl{rep}", bufs=2, space="PSUM") as plp,
                    tc.tile_pool(name=f"pv{rep}", bufs=2, space="PSUM") as pvp,
                    tc.tile_pool(name=f"pa2_{rep}", bufs=1, space="PSUM") as pap,
                    tc.tile_pool(name=f"pn{rep}", bufs=2, space="PSUM") as pnp,
                    tc.tile_pool(name=f"pr2_{rep}", bufs=1, space="PSUM") as pr2p,
                ):
                    state = {"pn": None}

                    def emit_front(c):
                        tokc = tok_all[:, c * 128 : (c + 1) * 128]
                        pl = plp.tile([128, 512], F32, tag="pl", name=f"pl{c}_{rep}")
                        nc.tensor.matmul(
                            pl[:], lhsT=tokc, rhs=bq1_sb[:], start=True, stop=True
                        )
                        E = rsb.tile([128, 512], F32, tag="E", name=f"E{c}_{rep}")
                        nc.scalar.activation(out=E[:], in_=pl[:], func=AF.Exp)

                        pv = pvp.tile([128, 136], F32, tag="pv", name=f"pv{c}_{rep}")
                        nc.tensor.matmul(
                            pv[:], lhsT=tokc, rhs=bwv1_sb[:], start=True, stop=True
                        )
                        m4 = c // 8
                        pa2 = pap.tile([128, 136], F32, tag="pa2", name=f"pa2_{c}_{rep}")
                        nc.tensor.matmul(
                            pa2[:],
                            lhsT=zg2[
                                32 * m4 : 32 * m4 + 8,
                                (c % 8) * 128 : (c % 8 + 1) * 128,
                            ],
                            rhs=exp8rep_sb[32 * m4 : 32 * m4 + 8, :],
                            start=True,
                            stop=True,
                            tile_position=(32 * m4, 0),
                        )
                        # sigmoid via exp to stay inside one ACT table set
                        easig = rsb.tile([128, 136], F32, tag="easig", name=f"ea{c}_{rep}")
                        nc.scalar.activation(
                            out=easig[:], in_=pa2[:], func=AF.Exp, scale=-1.0
                        )
                        asig = rsb.tile([128, 136], F32, tag="asig", name=f"as{c}_{rep}")
                        nc.vector.tensor_scalar_add(
                            out=asig[:], in0=easig[:], scalar1=1.0
                        )
                        nc.vector.reciprocal(out=asig[:], in_=asig[:])

                        vals = rsb.tile([128, 136], F32, tag="vals", name=f"va{c}_{rep}")
                        nc.vector.scalar_tensor_tensor(
                            out=vals[:],
                            in0=asig[:],
                            scalar=1e-6,
                            in1=pv[:],
                            op0=ALU.add,
                            op1=ALU.mult,
                        )
                        vr = vals[:].rearrange("p (n j) -> p n j", j=17)
                        ar = asig[:].rearrange("p (n j) -> p n j", j=17)
                        nc.vector.tensor_scalar_add(
                            out=vr[:, :, 16:17], in0=ar[:, :, 16:17], scalar1=1e-6
                        )
                        return E, vals

                    def emit_numer(c, E, vals):
                        even = c % 2 == 0
                        if even:
                            state["pn"] = pnp.tile(
                                [17, 64], F32, tag="pn", name=f"pn{c}_{rep}"
                            )
                        pn = state["pn"]
                        for n in range(8):
                            nc.tensor.matmul(
                                pn[:],
                                lhsT=vals[:, n * 17 : (n + 1) * 17],
                                rhs=E[:, n * 64 : (n + 1) * 64],
                                start=(even and n == 0),
                                stop=((not even) and n == 7),
                                skip_group_check=True,
                            )
                        return pn

                    def emit_r2(inst, pn):
                        acc_sb = small.tile([17, 64], F32, tag="acc", name=f"ac{inst}_{rep}")
                        nc.vector.tensor_copy(out=acc_sb[:], in_=pn[:])
                        pT = pr2p.tile([64, 17], F32, tag="r2", name=f"pT{inst}_{rep}")
                        nc.tensor.transpose(
                            out=pT[:], in_=acc_sb[:], identity=ident_sb[0:17, 0:17]
                        )
                        recd = small.tile([64, 1], F32, tag="recd", name=f"rc{inst}_{rep}")
                        nc.vector.reciprocal(out=recd[:], in_=pT[:, 16:17])
                        p1i = small.tile([64, 16], F32, tag="p1i", name=f"p1i{inst}_{rep}")
                        nc.vector.tensor_scalar_mul(
                            out=p1i[:], in0=pT[:, 0:16], scalar1=recd[:]
                        )
                        pP = pr2p.tile([16, 64], F32, tag="r2", name=f"pP{inst}_{rep}")
                        nc.tensor.transpose(
                            out=pP[:], in_=p1i[:], identity=ident_sb[0:64, 0:64]
                        )
                        p1T = small.tile([16, 64], F32, tag="p1T", name=f"p1T{inst}_{rep}")
                        nc.vector.tensor_copy(out=p1T[:], in_=pP[:])

                        # a1 = sigmoid(p1 @ wact1 + bact1) via a DVE dot in the
                        # o-on-partitions orientation, folded multiplicatively
                        # into the value matrix (no Ln -> single ACT table).
                        z1t = small.tile([64, 16], F32, tag="z1t", name=f"z1t{inst}_{rep}")
                        nc.vector.tensor_mul(
                            out=z1t[:], in0=p1i[:], in1=wact1rep_sb[:]
                        )
                        a1 = small.tile([64, 1], F32, tag="a1", name=f"a1_{inst}_{rep}")
                        nc.vector.reduce_sum(
                            out=a1[:], in_=z1t[:], axis=mybir.AxisListType.X
                        )
                        nc.scalar.activation(
                            out=a1[:],
                            in_=a1[:],
                            func=AF.Exp,
                            scale=-1.0,
                            bias=nbact1rep_sb[:],
                        )
                        nc.vector.tensor_scalar_add(out=a1[:], in0=a1[:], scalar1=1.0)
                        nc.vector.reciprocal(out=a1[:], in_=a1[:])
                        nc.vector.tensor_scalar_add(
                            out=a1[:], in0=a1[:], scalar1=1e-6
                        )
                        pv2 = small.tile([64, 17], F32, tag="pv2", name=f"pv2_{inst}_{rep}")
                        nc.vector.tensor_scalar_mul(
                            out=pv2[:, 0:16], in0=p1i[:], scalar1=a1[:]
                        )
                        nc.vector.tensor_copy(out=pv2[:, 16:17], in_=a1[:])

                        pL2 = pr2p.tile([64, NCLS], F32, tag="r2", name=f"pL2_{inst}_{rep}")
                        nc.tensor.matmul(
                            pL2[:], lhsT=p1T[:], rhs=q2sT_sb[:], start=True, stop=True
                        )
                        E2 = small.tile([64, NCLS], F32, tag="E2", name=f"E2_{inst}_{rep}")
                        nc.scalar.activation(out=E2[:], in_=pL2[:], func=AF.Exp)

                        pnd = pr2p.tile([NCLS, 17], F32, tag="r2", name=f"pnd{inst}_{rep}")
                        nc.tensor.matmul(
                            pnd[:], lhsT=E2[:], rhs=pv2[:], start=True, stop=True
                        )
                        recd2 = small.tile([NCLS, 1], F32, tag="recd2", name=f"rd{inst}_{rep}")
                        nc.vector.reciprocal(out=recd2[:], in_=pnd[:, 16:17])
                        p2 = small.tile([NCLS, 16], F32, tag="p2", name=f"p2_{inst}_{rep}")
                        nc.vector.tensor_scalar_mul(
                            out=p2[:], in0=pnd[:, 0:16], scalar1=recd2[:]
                        )
                        zt = small.tile([NCLS, 16], F32, tag="zt", name=f"zt{inst}_{rep}")
                        nc.vector.tensor_mul(out=zt[:], in0=p2[:], in1=wact2rep_sb[:])
                        z2 = small.tile([NCLS, 1], F32, tag="z2", name=f"z2_{inst}_{rep}")
                        nc.vector.reduce_sum(
                            out=z2[:], in_=zt[:], axis=mybir.AxisListType.X
                        )
                        ez2 = small.tile([NCLS, 1], F32, tag="ez2", name=f"ez{inst}_{rep}")
                        nc.scalar.activation(
                            out=ez2[:],
                            in_=z2[:],
                            func=AF.Exp,
                            scale=-1.0,
                            bias=nbact2rep_sb[:],
                        )
                        nc.vector.tensor_scalar_add(
                            out=ez2[:], in0=ez2[:], scalar1=1.0
                        )
                        nc.vector.reciprocal(
                            out=outcls_sb[:, inst : inst + 1], in_=ez2[:]
                        )

                    pending = None
                    for c in range(NCH if phases >= 2 else 0):
                        front = emit_front(c)
                        if pending is not None:
                            pc, pE, pvals = pending
                            pn = emit_numer(pc, pE, pvals)
                            if pc % 2 == 1 and phases >= 3:
                                emit_r2(pc // 2, pn)
                        pending = (c,) + front
                    if pending is not None:
                        pc, pE, pvals = pending
                        pn = emit_numer(pc, pE, pvals)
                        if pc % 2 == 1 and phases >= 3:
                            emit_r2(pc // 2, pn)

            nc.sync.dma_start(
                out=out_cls.rearrange("i c -> c i"), in_=outcls_sb[:]
            )

    nc.compile()
    return nc


def _get_nc():
    if "nc" not in _CACHE:
        _CACHE["nc"] = _build_nc()
    return _CACHE["nc"]


def _wrap_idx(sidx):
    # ap_gather index layout: index j lives at partition j%16, column j//16.
    return np.ascontiguousarray(sidx.reshape(-1, 16).T.astype(np.int16))


def host_prep(inputs):
    """Build the per-core input maps (all numpy, host-side weight folding)."""
    f8 = np.float64
    w_pos = np.asarray(inputs["w_pos"], f8)          # (16, 18)
    W16 = w_pos[:, :16]
    w_d = w_pos[:, 16] - w_pos[:, 17]                # (16,)
    b_pos = np.asarray(inputs["b_pos"], f8)
    w_vote = np.asarray(inputs["w_vote"], f8)        # (8, 16, 32)
    b_vote = np.asarray(inputs["b_vote"], f8)        # (8, 16)
    Wp = np.asarray(inputs["w_poses"], f8).reshape(NCAPS, DCAP, CIN)
    b_poses = np.asarray(inputs["b_poses"], f8).reshape(NCAPS, DCAP)

    Weff = np.stack([W16 @ w_vote[n] @ Wp[n] for n in range(NCAPS)])  # (8,16,1280)
    beff = np.stack(
        [W16 @ (w_vote[n] @ b_poses[n] + b_vote[n]) + b_pos for n in range(NCAPS)]
    )                                                                  # (8,16)
    Weff = Weff.reshape(128, CIN)
    beff = beff.reshape(128)

    ss = np.arange(S)
    r = ((ss // WF) - (ss % WF)) / 128.0
    pegrid = (np.tile(w_d, NCAPS)[:, None] * r[None, :] + beff[:, None]).astype(
        np.float32
    )

    Q1s = np.asarray(inputs["Q1"], f8) / 4.0         # (64, 16)
    BQ1 = np.zeros((128, 512), np.float32)
    for n in range(NCAPS):
        BQ1[n * 16 : (n + 1) * 16, n * 64 : (n + 1) * 64] = Q1s.T
    Wv1 = np.asarray(inputs["Wv1"], f8)
    BWV1 = np.zeros((128, 136), np.float32)
    for n in range(NCAPS):
        BWV1[n * 16 : (n + 1) * 16, n * 17 : n * 17 + 16] = Wv1
    EXP8REP = np.zeros((128, 136), np.float32)
    for m in range(4):
        for n in range(NCAPS):
            EXP8REP[32 * m + n, n * 17 : (n + 1) * 17] = 1.0

    consts = dict(
        pegrid=pegrid,
        weffT=np.ascontiguousarray(Weff.T).astype(BF16_NP),
        waT=np.ascontiguousarray(np.asarray(inputs["w_acts"], f8).T).astype(BF16_NP),
        bacts=np.asarray(inputs["b_acts"], np.float32).reshape(8, 1),
        bq1=BQ1,
        bwv1=BWV1,
        exp8rep=EXP8REP,
        q2sT=np.ascontiguousarray((np.asarray(inputs["Q2"], f8) / 4.0).T).astype(
            np.float32
        ),
        wact1rep=np.tile(
            np.asarray(inputs["wact1"], np.float32).reshape(1, 16), (64, 1)
        ),
        wact2rep=np.tile(
            np.asarray(inputs["wact2"], np.float32).reshape(1, 16), (NCLS, 1)
        ),
        nbact1rep=np.full(
            (64, 1), -float(np.asarray(inputs["bact1"])), np.float32
        ),
        nbact2rep=np.full(
            (NCLS, 1), -float(np.asarray(inputs["bact2"])), np.float32
        ),
        ident=np.eye(128, dtype=np.float32),
    )

    feats = np.asarray(inputs["feature_output"])     # (8, 1280, 64, 64) f32
    coords = np.asarray(inputs["point_coords"])      # (8, 16, 2, 256) int32
    mask = np.asarray(inputs["point_mask"])          # (8, 16, 256) bool

    in_maps = []
    for b in range(B):
        y = np.clip(coords[b, :, 0, :], 0, HF - 1).astype(np.int64)
        x = np.clip(coords[b, :, 1, :], 0, WF - 1).astype(np.int64)
        sidx = (y * WF + x).reshape(NPTS)
        zidx = sidx.copy()
        mb = mask[b].reshape(NPTS)
        zidx[~mb] = S  # masked points read the -1e4 z slot -> ~zero weight
        m = dict(consts)
        m["feat"] = np.ascontiguousarray(
            feats[b].reshape(CIN, S).astype(BF16_NP)
        )
        m["gidx"] = np.tile(_wrap_idx(sidx), (8, 1))
        # z-gather: even GPSIMD core 2m handles point quarter m
        aidx = np.zeros((128, NPTS // 64), np.int16)
        for mq in range(4):
            aidx[32 * mq : 32 * mq + 16, :] = _wrap_idx(
                zidx[mq * (NPTS // 4) : (mq + 1) * (NPTS // 4)]
            )
        m["aidx"] = aidx
        in_maps.append(m)
    return in_maps


def kernel(**inputs):
    nc = _get_nc()
    in_maps = host_prep(inputs)
    res = bass_utils.run_bass_kernel_spmd(nc, in_maps, core_ids=list(range(B)))
    out = np.stack([np.asarray(res.results[b]["out_cls"]) for b in range(B)])
    return out.astype(np.float32)



# revision 25
# speedup vs baseline: 12.4774x; 12.4774x over previous
"""Trainium2 Bass kernel for nn_CapsuleModel2 (capsule routing head).

Strategy (data-parallel, one image per NeuronCore, 8 cores):

Host-side algebraic folding (unchanged from v1):
  The per-pixel chain  1x1conv(poses) -> per-capsule vote conv ->
  positional-encoding linear  collapses into one effective matmul
     tokens_grid[(n,v), s] = Weff @ feat + pegrid
  with pegrid the host-precomputed rank-1 positional table.

Device pipeline per core (one image), v2:
  G:  tokens_grid = WeffT.T @ feat + pegrid   [128,4096] f32 (DVE add)
      z_grid = waT.T @ feat (raw logits, bias folded later)  [8,4096]
  H:  two ap_gathers (GPSIMD): raw z at points (zg2), tokens at points
      (tok_all). Acts sigmoid is applied ONCE post-gather:
      sg2 = 1/(1+exp(-z-bacts)) + 1e-6   [128,1024] bf16
  R1: per 128-point chunk: logits L = tok.T @ bq1 (tokens f32 as PE
      weights, bf16 moving consts), E = exp(L) bf16;
      vals = sg * (tok.T @ bwv1) bf16; numerator matmuls in the
      [o=64, j=17] orientation so all 16 instances accumulate into ONE
      psum bank [128, 17*8] and R2 runs fully batched.
  R2: batched across all 16 instances on [128,136]/[19,272] tiles:
      normalize, a1 sigmoid, Q2 attention, class sigmoid -> out [19,16].

All inputs are packed into 4 DRAM tensors (feat, cf32, cbf16, cidx) to
minimize per-dispatch argument overhead.
"""

import sys

for _p in ("/opt/trn_rl_repo",):
    if _p not in sys.path:
        sys.path.insert(0, _p)

import numpy as np
import ml_dtypes

import concourse.bacc as bacc
import concourse.tile as tile
from concourse import mybir
from concourse import bass_utils

AF = mybir.ActivationFunctionType
ALU = mybir.AluOpType
F32 = mybir.dt.float32
F32R = mybir.dt.float32r
BF16 = mybir.dt.bfloat16
I16 = mybir.dt.int16
BF16_NP = ml_dtypes.bfloat16

B, I, P = 8, 16, 256
CIN = 1280
NCAPS, DCAP, DV = 8, 32, 16
HF = WF = 64
S = HF * WF              # 4096 grid positions
NPTS = I * P             # 4096 gathered points
NOUT1, NCLS = 64, 19
KT = CIN // 128          # 10 contraction tiles
HALF = S // 2
ZW = S + 8               # z grid padded with a masked-point slot
NCH = 32                 # routing-1 chunks of 128 points

# --- cbf16 blob column offsets (bf16) ---
OB_WEFF = 0                      # [128, KT*128] weffT
OB_WA = OB_WEFF + KT * 128       # [128, KT*8]   waT
OB_BQ1 = OB_WA + KT * 8          # [128, 512]    blockdiag Q1/4
OB_BWV = OB_BQ1 + 512            # [128, 136]    blockdiag Wv1 (col16=0)
OB_E8 = OB_BWV + 136             # [128, 136]    z-replication matrix
OB_Q2 = OB_E8 + 136              # [16, 20]      (Q2/4).T zero-padded to 20
OB_IDT = OB_Q2 + 20              # [128, 128]    identity (transposes)
W_BF16 = OB_IDT + 128            # 2292

# --- cf32 blob column offsets (f32) ---
OF_PEG = 0                       # [128, 4096] pegrid
OF_W1R = OF_PEG + S              # [128, 136]  wact1 tiled per block, col16=0
OF_W2R = OF_W1R + 136            # [19, 272]   wact2 tiled per block, col16=0
OF_NB1 = OF_W2R + 272            # [128, 1]    -bact1
OF_NB2 = OF_NB1 + 1              # [19, 1]     -bact2
OF_NBG = OF_NB2 + 1              # [128, 1]    row 32m+n = -bacts[n]
W_F32 = OF_NBG + 1               # 4507

# --- cidx blob column offsets (i16) ---
OI_G = 0                         # [128, 256] token gather indices
OI_A = 256                       # [128, 64]  z gather indices
W_IDX = 320

_CACHE = {}


def _build_nc(repeat=1, gather_calls=1, phases=4, r2stage=6):
    nc = bacc.Bacc("TRN2", target_bir_lowering=False, debug=False, num_devices=8)

    feat = nc.dram_tensor("feat", [CIN, S], BF16, kind="ExternalInput").ap()
    cf32 = nc.dram_tensor("cf32", [128, W_F32], F32, kind="ExternalInput").ap()
    cbf16 = nc.dram_tensor("cbf16", [128, W_BF16], BF16, kind="ExternalInput").ap()
    cidx = nc.dram_tensor("cidx", [128, W_IDX], I16, kind="ExternalInput").ap()
    out_cls = nc.dram_tensor("out_cls", [I, NCLS], F32, kind="ExternalOutput").ap()

    with tile.TileContext(nc) as tc:
        with (
            tc.tile_pool(name="cons", bufs=1) as cons,
            tc.tile_pool(name="grid", bufs=1) as grid,
            tc.tile_pool(name="feats", bufs=4) as feats,
            tc.tile_pool(name="rsb", bufs=3) as rsb,
            tc.tile_pool(name="small", bufs=1) as small,
        ):
            # ---- constants to SBUF (scalar queue; feat uses sync queue) ----
            cb = cons.tile([128, W_BF16], BF16)
            nc.gpsimd.dma_start(out=cb[:], in_=cbf16)
            ci = cons.tile([128, W_IDX], I16)
            nc.gpsimd.dma_start(out=ci[:], in_=cidx)
            cf = cons.tile([128, W_F32], F32)
            for jq in range(2):
                half = W_F32 // 2
                lo = jq * half
                hi = W_F32 if jq == 1 else half
                nc.gpsimd.dma_start(out=cf[:, lo:hi], in_=cf32[:, lo:hi])

            # ---- persistent grid tensors ----
            tokens_sb = grid.tile([128, S], F32)
            z_rep = grid.tile([128, ZW], F32)
            tok_all = grid.tile([128, NPTS], F32)
            tok16 = grid.tile([128, NPTS], BF16)
            zg2 = grid.tile([128, NPTS // 4], F32)
            sg2 = grid.tile([128, NPTS // 4], BF16)
            outcls_sb = grid.tile([NCLS, I], F32)
            nc.vector.memset(z_rep[:, :], 0.0)
            # masked points gather this slot: sigmoid(-30)+1e-6 ~= 1e-6
            nc.vector.memset(z_rep[0:8, S:ZW], -30.0)

            for rep in range(repeat):
                # ================= phase G: grid matmuls =================
                with tc.tile_pool(name=f"pg{rep}", bufs=1, space="PSUM") as pg:
                    for h in range(2):
                        pms = [
                            pg.tile([128, 512], F32, tag=f"pm{nn}", name=f"pm{nn}_{h}_{rep}")
                            for nn in range(4)
                        ]
                        pas = [
                            pg.tile([8, 512], F32, tag=f"pa{nn}", name=f"pa{nn}_{h}_{rep}")
                            for nn in range(4)
                        ]
                        for k in range(KT):
                            ft = feats.tile([128, HALF], BF16, tag="feat")
                            nc.sync.dma_start(
                                out=ft[:],
                                in_=feat[k * 128 : (k + 1) * 128, h * HALF : (h + 1) * HALF],
                            )
                            for nn in range(4):
                                nc.tensor.matmul(
                                    pms[nn][:],
                                    lhsT=cb[:, OB_WEFF + k * 128 : OB_WEFF + (k + 1) * 128],
                                    rhs=ft[:, nn * 512 : (nn + 1) * 512],
                                    start=(k == 0),
                                    stop=(k == KT - 1),
                                )
                            for nn in range(4):
                                nc.tensor.matmul(
                                    pas[nn][:],
                                    lhsT=cb[:, OB_WA + k * 8 : OB_WA + (k + 1) * 8],
                                    rhs=ft[:, nn * 512 : (nn + 1) * 512],
                                    start=(k == 0),
                                    stop=(k == KT - 1),
                                )
                        for nn in range(4):
                            off = h * HALF + nn * 512
                            nc.vector.tensor_add(
                                out=tokens_sb[:, off : off + 512],
                                in0=pms[nn][:],
                                in1=cf[:, OF_PEG + off : OF_PEG + off + 512],
                            )
                            # raw z logits (bias applied post-gather)
                            nc.scalar.activation(
                                out=z_rep[0:8, off : off + 512],
                                in_=pas[nn][:],
                                func=AF.Identity,
                                scale=1.0,
                            )

                # replicate z rows 0:8 to partition bases 32/64/96
                for mq in range(1, 4):
                    nc.sync.dma_start(
                        out=z_rep[32 * mq : 32 * mq + 8, :], in_=z_rep[0:8, :]
                    )

                # ================= phase H: gathers ======================
                if phases < 2:
                    nc.vector.memset(outcls_sb[:, :], 0.0)
                    continue
                nc.gpsimd.ap_gather(
                    zg2[:],
                    z_rep[:],
                    ci[:, OI_A : OI_A + 64],
                    channels=128,
                    num_elems=ZW,
                    d=1,
                    num_idxs=NPTS // 4,
                )
                QI = NPTS // gather_calls
                for q in range(gather_calls):
                    nc.gpsimd.ap_gather(
                        tok_all[:, q * QI : (q + 1) * QI],
                        tokens_sb[:],
                        ci[:, OI_G + q * (QI // 16) : OI_G + (q + 1) * (QI // 16)],
                        channels=128,
                        num_elems=S,
                        d=1,
                        num_idxs=QI,
                    )
                for bq in range(4):
                    nc.vector.tensor_copy(
                        out=tok16[:, bq * 1024 : (bq + 1) * 1024],
                        in_=tok_all[:, bq * 1024 : (bq + 1) * 1024],
                    )
                # sg2 = sigmoid(z + bacts) + 1e-6, batched once (overlaps
                # the token gather on Pool)
                eg = grid.tile([128, NPTS // 4], F32, name=f"eg{rep}")
                nc.scalar.activation(
                    out=eg[:], in_=zg2[:], func=AF.Exp, scale=-1.0,
                    bias=cf[:, OF_NBG : OF_NBG + 1],
                )
                nc.vector.tensor_scalar_add(out=eg[:], in0=eg[:], scalar1=1.0)
                nc.vector.reciprocal(out=eg[:], in_=eg[:])
                nc.vector.tensor_scalar_add(out=sg2[:], in0=eg[:], scalar1=1e-6)

                # ================= phase R1: routing 1 ===================
                if phases < 3:
                    nc.vector.memset(outcls_sb[:, :], 0.0)
                    continue
                with (
                    tc.tile_pool(name=f"pn{rep}", bufs=1, space="PSUM") as pnp,
                ):
                    pn_all = pnp.tile([128, 136], F32, name=f"pnall_{rep}")

                    with (
                        tc.tile_pool(name=f"pl{rep}", bufs=2, space="PSUM") as plp,
                        tc.tile_pool(name=f"pv{rep}", bufs=2, space="PSUM") as pvp,
                    ):
                        def emit_front(c):
                            tokc = tok16[:, c * 128 : (c + 1) * 128]
                            m4 = c // 8
                            pl = plp.tile([128, 512], F32, tag="pl", name=f"pl{c}_{rep}")
                            nc.tensor.matmul(
                                pl[:], lhsT=tokc,
                                rhs=cb[:, OB_BQ1 : OB_BQ1 + 512],
                                start=True, stop=True,
                            )
                            E = rsb.tile([128, 512], BF16, tag="E", name=f"E{c}_{rep}")
                            nc.scalar.activation(out=E[:], in_=pl[:], func=AF.Exp)

                            pvpa = pvp.tile([128, 272], F32, tag="pv", name=f"pv{c}_{rep}")
                            nc.tensor.matmul(
                                pvpa[:, 0:136], lhsT=tokc,
                                rhs=cb[:, OB_BWV : OB_BWV + 136],
                                start=True, stop=True,
                            )
                            nc.tensor.matmul(
                                pvpa[:, 136:272],
                                lhsT=sg2[
                                    32 * m4 : 32 * m4 + 8,
                                    (c % 8) * 128 : (c % 8 + 1) * 128,
                                ],
                                rhs=cb[32 * m4 : 32 * m4 + 8, OB_E8 : OB_E8 + 136],
                                start=True, stop=True,
                                tile_position=(32 * m4, 0),
                            )
                            asig = rsb.tile([128, 136], BF16, tag="asig", name=f"as{c}_{rep}")
                            nc.vector.tensor_copy(out=asig[:], in_=pvpa[:, 136:272])
                            vals = rsb.tile([128, 136], BF16, tag="vals", name=f"va{c}_{rep}")
                            nc.vector.tensor_mul(
                                out=vals[:], in0=pvpa[:, 0:136], in1=asig[:]
                            )
                            vr = vals[:].rearrange("p (n j) -> p n j", j=17)
                            ar = asig[:].rearrange("p (n j) -> p n j", j=17)
                            nc.vector.tensor_copy(
                                out=vr[:, :, 16:17], in_=ar[:, :, 16:17]
                            )
                            return E, vals

                        def emit_numer(c, E, vals):
                            i = c // 2
                            po = 64 * (i % 2)
                            co = 17 * (i // 2)
                            for n in range(8):
                                nc.tensor.matmul(
                                    pn_all[po : po + 64, co : co + 17],
                                    lhsT=E[:, n * 64 : (n + 1) * 64],
                                    rhs=vals[:, n * 17 : (n + 1) * 17],
                                    start=(c % 2 == 0 and n == 0),
                                    stop=(c % 2 == 1 and n == 7),
                                    skip_group_check=True,
                                )

                        pending = None
                        for c in range(NCH):
                            front = emit_front(c)
                            if pending is not None:
                                emit_numer(*pending)
                            pending = (c,) + front
                        emit_numer(*pending)

                    # ================= phase R2: batched =================
                    if phases < 4:
                        nc.vector.memset(outcls_sb[:, :], 0.0)
                        continue
                    with (
                        tc.tile_pool(name=f"pt{rep}", bufs=1, space="PSUM") as ptp,
                        tc.tile_pool(name=f"pq{rep}", bufs=1, space="PSUM") as pqp,
                    ):
                        pnS = small.tile([128, 136], F32, name=f"pnS_{rep}")
                        nc.vector.tensor_copy(out=pnS[:], in_=pn_all[:])
                        pnV = pnS[:].rearrange("p (i j) -> p i j", j=17)

                        recd = small.tile([128, 8], F32, name=f"recd_{rep}")
                        nc.vector.reciprocal(out=recd[:], in_=pnV[:, :, 16:17])

                        z1t = small.tile([128, 136], F32, name=f"z1t_{rep}")
                        nc.vector.tensor_mul(
                            out=z1t[:], in0=pnS[:], in1=cf[:, OF_W1R : OF_W1R + 136]
                        )
                        s1 = small.tile([128, 8], F32, name=f"s1_{rep}")
                        nc.vector.reduce_sum(
                            out=s1[:],
                            in_=z1t[:].rearrange("p (i j) -> p i j", j=17),
                            axis=mybir.AxisListType.X,
                        )
                        z1 = small.tile([128, 8], F32, name=f"z1_{rep}")
                        nc.vector.tensor_mul(out=z1[:], in0=s1[:], in1=recd[:])
                        a1e = small.tile([128, 8], F32, name=f"a1e_{rep}")
                        nc.scalar.activation(
                            out=a1e[:], in_=z1[:], func=AF.Exp, scale=-1.0,
                            bias=cf[:, OF_NB1 : OF_NB1 + 1],
                        )
                        nc.vector.tensor_scalar_add(out=a1e[:], in0=a1e[:], scalar1=1.0)
                        nc.vector.reciprocal(out=a1e[:], in_=a1e[:])
                        nc.vector.tensor_scalar_add(out=a1e[:], in0=a1e[:], scalar1=1e-6)
                        sc = small.tile([128, 8], F32, name=f"sc_{rep}")
                        nc.vector.tensor_mul(out=sc[:], in0=recd[:], in1=a1e[:])

                        p1b = small.tile([128, 128], BF16, name=f"p1b_{rep}")
                        # 20-col blocks keep pnd's rhs slices 4B-aligned
                        pv2 = small.tile([128, 160], BF16, name=f"pv2_{rep}")
                        for k in range(8):
                            nc.vector.tensor_scalar_mul(
                                out=p1b[:, 16 * k : 16 * k + 16],
                                in0=pnS[:, 17 * k : 17 * k + 16],
                                scalar1=recd[:, k : k + 1],
                            )
                            nc.vector.tensor_scalar_mul(
                                out=pv2[:, 20 * k : 20 * k + 16],
                                in0=pnS[:, 17 * k : 17 * k + 16],
                                scalar1=sc[:, k : k + 1],
                            )
                        pv2V = pv2[:].rearrange("p (i j) -> p i j", j=20)
                        nc.vector.tensor_copy(out=pv2V[:, :, 16:17], in_=a1e[:])

                        if r2stage < 2:
                            nc.vector.memset(outcls_sb[:, :], 0.0)
                            continue
                        # transposes: p1 [128,16] blocks -> [16,128] (2 inst)
                        pts = [
                            ptp.tile([16, 512], BF16, tag=f"pt{t}", name=f"pt{t}_{rep}")
                            for t in range(2)
                        ]
                        for k in range(8):
                            nc.tensor.transpose(
                                out=pts[k // 4][:, 128 * (k % 4) : 128 * (k % 4) + 128],
                                in_=p1b[:, 16 * k : 16 * k + 16],
                                identity=cb[:, OB_IDT : OB_IDT + 128],
                            )
                        pTS = small.tile([16, 1024], BF16, name=f"pTS_{rep}")
                        for t in range(2):
                            nc.vector.tensor_copy(
                                out=pTS[:, 512 * t : 512 * t + 512], in_=pts[t][:]
                            )

                        if r2stage < 3:
                            nc.vector.memset(outcls_sb[:, :], 0.0)
                            continue
                        pL2 = pqp.tile([128, 160], F32, name=f"pL2_{rep}")
                        for k in range(8):
                            nc.tensor.matmul(
                                pL2[:, 20 * k : 20 * k + 20],
                                lhsT=pTS[:, 128 * k : 128 * k + 128],
                                rhs=cb[0:16, OB_Q2 : OB_Q2 + 20],
                                start=True, stop=True,
                            )
                        E2 = small.tile([128, 160], BF16, name=f"E2_{rep}")
                        nc.scalar.activation(out=E2[:], in_=pL2[:], func=AF.Exp)

                        if r2stage < 4:
                            nc.vector.memset(outcls_sb[:, :], 0.0)
                            continue
                        # Odd instances (partitions 64:128) are DMA-shifted to
                        # partition base 0: back-to-back matmuls whose operand
                        # base partition flips 0<->64 lock up the PE (probed).
                        E2lo = small.tile([64, 160], BF16, name=f"E2lo_{rep}")
                        nc.sync.dma_start(out=E2lo[:], in_=E2[64:128, :])
                        pv2lo = small.tile([64, 160], BF16, name=f"pv2lo_{rep}")
                        nc.sync.dma_start(out=pv2lo[:], in_=pv2[64:128, :])
                        pnd = pqp.tile([NCLS, 272], F32, name=f"pnd_{rep}")
                        for i in range(I):
                            k, hh = i // 2, i % 2
                            Esrc = E2 if hh == 0 else E2lo
                            vsrc = pv2 if hh == 0 else pv2lo
                            nc.tensor.matmul(
                                pnd[:, 17 * i : 17 * i + 17],
                                lhsT=Esrc[0:64, 20 * k : 20 * k + 19],
                                rhs=vsrc[0:64, 20 * k : 20 * k + 17],
                                start=True, stop=True,
                            )
                        pndS = small.tile([NCLS, 272], F32, name=f"pndS_{rep}")
                        nc.vector.tensor_copy(out=pndS[:], in_=pnd[:])
                        if r2stage < 5:
                            nc.vector.memset(outcls_sb[:, :], 0.0)
                            continue
                        pndV = pndS[:].rearrange("p (i j) -> p i j", j=17)
                        recd2 = small.tile([NCLS, 16], F32, name=f"recd2_{rep}")
                        nc.vector.reciprocal(out=recd2[:], in_=pndV[:, :, 16:17])
                        z2t = small.tile([NCLS, 272], F32, name=f"z2t_{rep}")
                        nc.vector.tensor_mul(
                            out=z2t[:], in0=pndS[:], in1=cf[0:NCLS, OF_W2R : OF_W2R + 272]
                        )
                        s2 = small.tile([NCLS, 16], F32, name=f"s2_{rep}")
                        nc.vector.reduce_sum(
                            out=s2[:],
                            in_=z2t[:].rearrange("p (i j) -> p i j", j=17),
                            axis=mybir.AxisListType.X,
                        )
                        z2 = small.tile([NCLS, 16], F32, name=f"z2_{rep}")
                        nc.vector.tensor_mul(out=z2[:], in0=s2[:], in1=recd2[:])
                        if r2stage < 6:
                            nc.vector.memset(outcls_sb[:, :], 0.0)
                            continue
                        ez2 = small.tile([NCLS, 16], F32, name=f"ez2_{rep}")
                        nc.scalar.activation(
                            out=ez2[:], in_=z2[:], func=AF.Exp, scale=-1.0,
                            bias=cf[0:NCLS, OF_NB2 : OF_NB2 + 1],
                        )
                        nc.vector.tensor_scalar_add(out=ez2[:], in0=ez2[:], scalar1=1.0)
                        nc.vector.reciprocal(out=outcls_sb[:], in_=ez2[:])

            nc.sync.dma_start(out=out_cls.rearrange("i c -> c i"), in_=outcls_sb[:])

    nc.compile()
    return nc


def _get_nc():
    if "nc" not in _CACHE:
        _CACHE["nc"] = _build_nc()
    return _CACHE["nc"]


def _wrap_idx(sidx):
    # ap_gather index layout: index j lives at partition j%16, column j//16.
    return np.ascontiguousarray(sidx.reshape(-1, 16).T.astype(np.int16))


def host_prep(inputs):
    """Build the per-core input maps (all numpy, host-side weight folding)."""
    f8 = np.float64
    w_pos = np.asarray(inputs["w_pos"], f8)          # (16, 18)
    W16 = w_pos[:, :16]
    w_d = w_pos[:, 16] - w_pos[:, 17]                # (16,)
    b_pos = np.asarray(inputs["b_pos"], f8)
    w_vote = np.asarray(inputs["w_vote"], f8)        # (8, 16, 32)
    b_vote = np.asarray(inputs["b_vote"], f8)        # (8, 16)
    Wp = np.asarray(inputs["w_poses"], f8).reshape(NCAPS, DCAP, CIN)
    b_poses = np.asarray(inputs["b_poses"], f8).reshape(NCAPS, DCAP)

    Weff = np.stack([W16 @ w_vote[n] @ Wp[n] for n in range(NCAPS)])  # (8,16,1280)
    beff = np.stack(
        [W16 @ (w_vote[n] @ b_poses[n] + b_vote[n]) + b_pos for n in range(NCAPS)]
    )
    Weff = Weff.reshape(128, CIN)
    beff = beff.reshape(128)

    ss = np.arange(S)
    r = ((ss // WF) - (ss % WF)) / 128.0
    pegrid = (np.tile(w_d, NCAPS)[:, None] * r[None, :] + beff[:, None]).astype(
        np.float32
    )

    Q1s = np.asarray(inputs["Q1"], f8) / 4.0         # (64, 16)
    BQ1 = np.zeros((128, 512), f8)
    for n in range(NCAPS):
        BQ1[n * 16 : (n + 1) * 16, n * 64 : (n + 1) * 64] = Q1s.T
    Wv1 = np.asarray(inputs["Wv1"], f8)
    BWV1 = np.zeros((128, 136), f8)
    for n in range(NCAPS):
        BWV1[n * 16 : (n + 1) * 16, n * 17 : n * 17 + 16] = Wv1
    EXP8REP = np.zeros((128, 136), f8)
    for m in range(4):
        for n in range(NCAPS):
            EXP8REP[32 * m + n, n * 17 : (n + 1) * 17] = 1.0

    # ---- cbf16 blob ----
    cbf16 = np.zeros((128, W_BF16), np.float64)
    weffT = Weff.T.reshape(KT, 128, 128).transpose(1, 0, 2).reshape(128, KT * 128)
    cbf16[:, OB_WEFF : OB_WEFF + KT * 128] = weffT
    waT = np.asarray(inputs["w_acts"], f8).T.reshape(KT, 128, 8)
    cbf16[:, OB_WA : OB_WA + KT * 8] = waT.transpose(1, 0, 2).reshape(128, KT * 8)
    cbf16[:, OB_BQ1 : OB_BQ1 + 512] = BQ1
    cbf16[:, OB_BWV : OB_BWV + 136] = BWV1
    cbf16[:, OB_E8 : OB_E8 + 136] = EXP8REP
    cbf16[0:16, OB_Q2 : OB_Q2 + NCLS] = (np.asarray(inputs["Q2"], f8) / 4.0).T
    cbf16[:, OB_IDT : OB_IDT + 128] = np.eye(128)
    cbf16 = cbf16.astype(BF16_NP)

    # ---- cf32 blob ----
    cf32 = np.zeros((128, W_F32), np.float32)
    cf32[:, OF_PEG : OF_PEG + S] = pegrid
    wact1 = np.asarray(inputs["wact1"], np.float64)
    w1row = np.tile(np.concatenate([wact1, [0.0]]), NCAPS)        # (136,)
    cf32[:, OF_W1R : OF_W1R + 136] = w1row[None, :]
    wact2 = np.asarray(inputs["wact2"], np.float64)
    w2row = np.tile(np.concatenate([wact2, [0.0]]), I)            # (272,)
    cf32[0:NCLS, OF_W2R : OF_W2R + 272] = w2row[None, :]
    cf32[:, OF_NB1] = -float(np.asarray(inputs["bact1"]))
    cf32[0:NCLS, OF_NB2] = -float(np.asarray(inputs["bact2"]))
    bacts = np.asarray(inputs["b_acts"], np.float64)
    nbg = np.zeros(128)
    for m in range(4):
        nbg[32 * m : 32 * m + 8] = -bacts
    cf32[:, OF_NBG] = nbg

    consts = dict(cf32=cf32, cbf16=cbf16)

    feats = np.asarray(inputs["feature_output"])     # (8, 1280, 64, 64) f32
    coords = np.asarray(inputs["point_coords"])      # (8, 16, 2, 256) int32
    mask = np.asarray(inputs["point_mask"])          # (8, 16, 256) bool

    in_maps = []
    for b in range(B):
        y = np.clip(coords[b, :, 0, :], 0, HF - 1).astype(np.int64)
        x = np.clip(coords[b, :, 1, :], 0, WF - 1).astype(np.int64)
        sidx = (y * WF + x).reshape(NPTS)
        zidx = sidx.copy()
        mb = mask[b].reshape(NPTS)
        zidx[~mb] = S  # masked points read the -30 z slot -> ~zero weight
        m = dict(consts)
        m["feat"] = np.ascontiguousarray(
            feats[b].reshape(CIN, S).astype(BF16_NP)
        )
        cidx = np.zeros((128, W_IDX), np.int16)
        cidx[:, OI_G : OI_G + 256] = np.tile(_wrap_idx(sidx), (8, 1))
        for mq in range(4):
            cidx[32 * mq : 32 * mq + 16, OI_A : OI_A + 64] = _wrap_idx(
                zidx[mq * (NPTS // 4) : (mq + 1) * (NPTS // 4)]
            )
        m["cidx"] = cidx
        in_maps.append(m)
    return in_maps


def kernel(**inputs):
    nc = _get_nc()
    in_maps = host_prep(inputs)
    res = bass_utils.run_bass_kernel_spmd(nc, in_maps, core_ids=list(range(B)))
    out = np.stack([np.asarray(res.results[b]["out_cls"]) for b in range(B)])
    return out.astype(np.float32)


# revision 27
# speedup vs baseline: 21.7592x; 1.7439x over previous
"""Trainium2 Bass kernel for nn_CapsuleModel2 (capsule routing head).

Strategy (data-parallel, one image per NeuronCore, 8 cores):

Host-side folding:
  1x1conv(poses) -> vote conv -> positional linear collapses into
     tokens[(n,v), j] = Weff @ feat_pts + pegrid_pts
  AND the point gather is hoisted to the host: feat columns arrive
  already permuted into point order (feat_pts[:, j] = feat[:, sidx[j]]),
  so the device computes tokens/acts DIRECTLY per point — no GPSIMD
  gather. Row 1280 of the feat input is a mask row (-30 for masked
  points) accumulated into the act logits via a 1-partition matmul.

Device pipeline per core (one image):
  G:  tokens[(n,v), j] (bf16, DVE add of psum+pegrid), act logits
      z[n, j] (+mask row), exp(-z-b) written straight into a
      4x32-partition-block quartered layout; 3 DVE ops finish
      sg = sigmoid+1e-6 as [128, 1024] bf16.
  R1: per 128-point chunk: L = tok.T @ blockdiag(Q1/4), E = exp(L) bf16;
      vals = sg * (tok.T @ blockdiag(Wv1)) bf16; numerator matmuls in
      the [o=64, j=17] orientation so all 16 instances accumulate into
      ONE psum bank [128, 17*8] and R2 runs fully batched.
  R2: batched across all 16 instances on [128,*]/[19,272] tiles:
      normalize, a1 sigmoid, Q2 attention, class sigmoid -> out [19,16].

Inputs are packed into 3 DRAM tensors (feat, cf32, cbf16) to minimize
per-dispatch argument overhead.

Hardware pitfall encoded here: back-to-back PE matmuls whose operand
base partition flips 0<->64 lock up the device (probed on HW), so the
R2 class-attention stage DMA-shifts odd instances to partition base 0.
"""

import sys

for _p in ("/opt/trn_rl_repo",):
    if _p not in sys.path:
        sys.path.insert(0, _p)

import numpy as np
import ml_dtypes

import concourse.bacc as bacc
import concourse.tile as tile
from concourse import mybir
from concourse import bass_utils

AF = mybir.ActivationFunctionType
ALU = mybir.AluOpType
F32 = mybir.dt.float32
BF16 = mybir.dt.bfloat16
BF16_NP = ml_dtypes.bfloat16

B, I, P = 8, 16, 256
CIN = 1280
NCAPS, DCAP, DV = 8, 32, 16
HF = WF = 64
S = HF * WF              # 4096 grid positions
NPTS = I * P             # 4096 points (== S by coincidence)
NOUT1, NCLS = 64, 19
KT = CIN // 128          # 10 contraction tiles
HALF = NPTS // 2
NCH = 32                 # routing-1 chunks of 128 points

# --- cbf16 blob column offsets (bf16) ---
OB_WEFF = 0                      # [128, KT*128] weffT
OB_WA = OB_WEFF + KT * 128       # [128, KT*8]   waT
OB_BQ1 = OB_WA + KT * 8          # [128, 512]    blockdiag Q1/4
OB_BWV = OB_BQ1 + 512            # [128, 136]    blockdiag Wv1 (col16=0)
OB_E8 = OB_BWV + 136             # [128, 136]    act-replication matrix
OB_Q2 = OB_E8 + 136              # [16, 20]      (Q2/4).T zero-padded
OB_IDT = OB_Q2 + 20              # [128, 128]    identity (transposes)
OB_ONE = OB_IDT + 128            # [1, 8]        ones (mask-row matmul)
W_BF16 = OB_ONE + 8              # 2300

# --- cf32 blob column offsets (f32; pegrid part is per-image) ---
OF_PEG = 0                       # [128, 4096] positional table at points
OF_W1R = OF_PEG + NPTS           # [128, 136]  wact1 tiled per block, col16=0
OF_W2R = OF_W1R + 136            # [19, 272]   wact2 tiled per block, col16=0
OF_NB1 = OF_W2R + 272            # [128, 1]    -bact1
OF_NB2 = OF_NB1 + 1              # [19, 1]     -bact2
OF_NBG = OF_NB2 + 1              # [128, 1]    row 32m+n = -bacts[n]
W_F32 = OF_NBG + 1               # 4507

_CACHE = {}


def _build_nc(repeat=1, phases=4):
    nc = bacc.Bacc("TRN2", target_bir_lowering=False, debug=False, num_devices=8)

    feat = nc.dram_tensor("feat", [CIN + 1, NPTS], BF16, kind="ExternalInput").ap()
    cf32 = nc.dram_tensor("cf32", [128, W_F32], F32, kind="ExternalInput").ap()
    cbf16 = nc.dram_tensor("cbf16", [128, W_BF16], BF16, kind="ExternalInput").ap()
    out_cls = nc.dram_tensor("out_cls", [I, NCLS], F32, kind="ExternalOutput").ap()

    with tile.TileContext(nc) as tc:
        with (
            tc.tile_pool(name="cons", bufs=1) as cons,
            tc.tile_pool(name="grid", bufs=1) as grid,
            tc.tile_pool(name="feats", bufs=4) as feats,
            tc.tile_pool(name="rsb", bufs=3) as rsb,
            tc.tile_pool(name="small", bufs=1) as small,
        ):
            # ---- constants via the gpsimd (SWDGE) queue; feat on sync ----
            cb = cons.tile([128, W_BF16], BF16)
            nc.gpsimd.dma_start(out=cb[:], in_=cbf16)
            cf = cons.tile([128, W_F32], F32)
            for jq in range(2):
                half = W_F32 // 2
                lo = jq * half
                hi = W_F32 if jq == 1 else half
                nc.gpsimd.dma_start(out=cf[:, lo:hi], in_=cf32[:, lo:hi])

            # ---- persistent tensors ----
            tokens_sb = grid.tile([128, NPTS], BF16)
            zq = grid.tile([128, NPTS // 4], F32)    # quartered act logits
            sg2 = grid.tile([128, NPTS // 4], BF16)  # sigmoid(z)+1e-6
            outcls_sb = grid.tile([NCLS, I], F32)
            nc.vector.memset(zq[:, :], 0.0)

            for rep in range(repeat):
                # ================= phase G: point matmuls ================
                with tc.tile_pool(name=f"pg{rep}", bufs=1, space="PSUM") as pg:
                    for h in range(2):
                        mrow = feats.tile([1, HALF], BF16, tag="mrow", name=f"mr{h}_{rep}")
                        nc.sync.dma_start(
                            out=mrow[:], in_=feat[CIN : CIN + 1, h * HALF : (h + 1) * HALF]
                        )
                        pms = [
                            pg.tile([128, 512], F32, tag=f"pm{nn}", name=f"pm{nn}_{h}_{rep}")
                            for nn in range(4)
                        ]
                        pas = [
                            pg.tile([8, 512], F32, tag=f"pa{nn}", name=f"pa{nn}_{h}_{rep}")
                            for nn in range(4)
                        ]
                        for k in range(KT):
                            ft = feats.tile([128, HALF], BF16, tag="feat")
                            nc.sync.dma_start(
                                out=ft[:],
                                in_=feat[k * 128 : (k + 1) * 128, h * HALF : (h + 1) * HALF],
                            )
                            for nn in range(4):
                                nc.tensor.matmul(
                                    pms[nn][:],
                                    lhsT=cb[:, OB_WEFF + k * 128 : OB_WEFF + (k + 1) * 128],
                                    rhs=ft[:, nn * 512 : (nn + 1) * 512],
                                    start=(k == 0),
                                    stop=(k == KT - 1),
                                )
                            for nn in range(4):
                                nc.tensor.matmul(
                                    pas[nn][:],
                                    lhsT=cb[:, OB_WA + k * 8 : OB_WA + (k + 1) * 8],
                                    rhs=ft[:, nn * 512 : (nn + 1) * 512],
                                    start=(k == 0),
                                    stop=False,
                                )
                        # mask row folds into the act logits
                        for nn in range(4):
                            nc.tensor.matmul(
                                pas[nn][:],
                                lhsT=cb[0:1, OB_ONE : OB_ONE + 8],
                                rhs=mrow[:, nn * 512 : (nn + 1) * 512],
                                start=False,
                                stop=True,
                            )
                        for nn in range(4):
                            off = h * HALF + nn * 512
                            q, loc = off // 1024, off % 1024
                            nc.vector.tensor_add(
                                out=tokens_sb[:, off : off + 512],
                                in0=pms[nn][:],
                                in1=cf[:, OF_PEG + off : OF_PEG + off + 512],
                            )
                            # exp(-z - bacts) straight into the quartered
                            # 32-row-block layout R1's broadcast matmul eats
                            nc.scalar.activation(
                                out=zq[32 * q : 32 * q + 8, loc : loc + 512],
                                in_=pas[nn][:],
                                func=AF.Exp,
                                scale=-1.0,
                                bias=cf[32 * q : 32 * q + 8, OF_NBG : OF_NBG + 1],
                            )

                if phases < 2:
                    nc.vector.memset(outcls_sb[:, :], 0.0)
                    continue

                # finish sg = 1/(1+exp(-z-b)) + 1e-6 on [128, 1024]
                nc.vector.tensor_scalar_add(out=zq[:], in0=zq[:], scalar1=1.0)
                nc.vector.reciprocal(out=zq[:], in_=zq[:])
                nc.vector.tensor_scalar_add(out=sg2[:], in0=zq[:], scalar1=1e-6)

                if phases < 3:
                    nc.vector.memset(outcls_sb[:, :], 0.0)
                    continue

                # ================= phase R1: routing 1 ===================
                with (
                    tc.tile_pool(name=f"pn{rep}", bufs=1, space="PSUM") as pnp,
                ):
                    pn_all = pnp.tile([128, 136], F32, name=f"pnall_{rep}")

                    with (
                        tc.tile_pool(name=f"pl{rep}", bufs=2, space="PSUM") as plp,
                        tc.tile_pool(name=f"pv{rep}", bufs=2, space="PSUM") as pvp,
                    ):
                        def emit_front(c):
                            tokc = tokens_sb[:, c * 128 : (c + 1) * 128]
                            m4 = c // 8
                            pl = plp.tile([128, 512], F32, tag="pl", name=f"pl{c}_{rep}")
                            nc.tensor.matmul(
                                pl[:], lhsT=tokc,
                                rhs=cb[:, OB_BQ1 : OB_BQ1 + 512],
                                start=True, stop=True,
                            )
                            E = rsb.tile([128, 512], BF16, tag="E", name=f"E{c}_{rep}")
                            nc.scalar.activation(out=E[:], in_=pl[:], func=AF.Exp)

                            pvpa = pvp.tile([128, 272], F32, tag="pv", name=f"pv{c}_{rep}")
                            nc.tensor.matmul(
                                pvpa[:, 0:136], lhsT=tokc,
                                rhs=cb[:, OB_BWV : OB_BWV + 136],
                                start=True, stop=True,
                            )
                            nc.tensor.matmul(
                                pvpa[:, 136:272],
                                lhsT=sg2[
                                    32 * m4 : 32 * m4 + 8,
                                    (c % 8) * 128 : (c % 8 + 1) * 128,
                                ],
                                rhs=cb[32 * m4 : 32 * m4 + 8, OB_E8 : OB_E8 + 136],
                                start=True, stop=True,
                                tile_position=(32 * m4, 0),
                            )
                            asig = rsb.tile([128, 136], BF16, tag="asig", name=f"as{c}_{rep}")
                            nc.vector.tensor_copy(out=asig[:], in_=pvpa[:, 136:272])
                            vals = rsb.tile([128, 136], BF16, tag="vals", name=f"va{c}_{rep}")
                            nc.vector.tensor_mul(
                                out=vals[:], in0=pvpa[:, 0:136], in1=asig[:]
                            )
                            vr = vals[:].rearrange("p (n j) -> p n j", j=17)
                            ar = asig[:].rearrange("p (n j) -> p n j", j=17)
                            nc.vector.tensor_copy(
                                out=vr[:, :, 16:17], in_=ar[:, :, 16:17]
                            )
                            return E, vals

                        def emit_numer(c, E, vals):
                            i = c // 2
                            po = 64 * (i % 2)
                            co = 17 * (i // 2)
                            for n in range(8):
                                nc.tensor.matmul(
                                    pn_all[po : po + 64, co : co + 17],
                                    lhsT=E[:, n * 64 : (n + 1) * 64],
                                    rhs=vals[:, n * 17 : (n + 1) * 17],
                                    start=(c % 2 == 0 and n == 0),
                                    stop=(c % 2 == 1 and n == 7),
                                    skip_group_check=True,
                                )

                        pending = None
                        for c in range(NCH):
                            front = emit_front(c)
                            if pending is not None:
                                emit_numer(*pending)
                            pending = (c,) + front
                        emit_numer(*pending)

                    # ================= phase R2: batched =================
                    if phases < 4:
                        nc.vector.memset(outcls_sb[:, :], 0.0)
                        continue
                    with (
                        tc.tile_pool(name=f"pt{rep}", bufs=1, space="PSUM") as ptp,
                        tc.tile_pool(name=f"pq{rep}", bufs=1, space="PSUM") as pqp,
                    ):
                        pnS = small.tile([128, 136], F32, name=f"pnS_{rep}")
                        nc.vector.tensor_copy(out=pnS[:], in_=pn_all[:])
                        pnV = pnS[:].rearrange("p (i j) -> p i j", j=17)

                        recd = small.tile([128, 8], F32, name=f"recd_{rep}")
                        nc.vector.reciprocal(out=recd[:], in_=pnV[:, :, 16:17])

                        z1t = small.tile([128, 136], F32, name=f"z1t_{rep}")
                        nc.vector.tensor_mul(
                            out=z1t[:], in0=pnS[:], in1=cf[:, OF_W1R : OF_W1R + 136]
                        )
                        s1 = small.tile([128, 8], F32, name=f"s1_{rep}")
                        nc.vector.reduce_sum(
                            out=s1[:],
                            in_=z1t[:].rearrange("p (i j) -> p i j", j=17),
                            axis=mybir.AxisListType.X,
                        )
                        z1 = small.tile([128, 8], F32, name=f"z1_{rep}")
                        nc.vector.tensor_mul(out=z1[:], in0=s1[:], in1=recd[:])
                        a1e = small.tile([128, 8], F32, name=f"a1e_{rep}")
                        nc.scalar.activation(
                            out=a1e[:], in_=z1[:], func=AF.Exp, scale=-1.0,
                            bias=cf[:, OF_NB1 : OF_NB1 + 1],
                        )
                        nc.vector.tensor_scalar_add(out=a1e[:], in0=a1e[:], scalar1=1.0)
                        nc.vector.reciprocal(out=a1e[:], in_=a1e[:])
                        nc.vector.tensor_scalar_add(out=a1e[:], in0=a1e[:], scalar1=1e-6)
                        sc = small.tile([128, 8], F32, name=f"sc_{rep}")
                        nc.vector.tensor_mul(out=sc[:], in0=recd[:], in1=a1e[:])

                        p1b = small.tile([128, 128], BF16, name=f"p1b_{rep}")
                        # 20-col blocks keep pnd's rhs slices 4B-aligned
                        pv2 = small.tile([128, 160], BF16, name=f"pv2_{rep}")
                        nc.vector.memset(pv2[:, :], 0.0)
                        for k in range(8):
                            nc.vector.tensor_scalar_mul(
                                out=p1b[:, 16 * k : 16 * k + 16],
                                in0=pnS[:, 17 * k : 17 * k + 16],
                                scalar1=recd[:, k : k + 1],
                            )
                            nc.vector.tensor_scalar_mul(
                                out=pv2[:, 20 * k : 20 * k + 16],
                                in0=pnS[:, 17 * k : 17 * k + 16],
                                scalar1=sc[:, k : k + 1],
                            )
                        pv2V = pv2[:].rearrange("p (i j) -> p i j", j=20)
                        nc.vector.tensor_copy(out=pv2V[:, :, 16:17], in_=a1e[:])

                        # transposes: p1 [128,16] blocks -> [16,128] (2 inst)
                        pts = [
                            ptp.tile([16, 512], BF16, tag=f"pt{t}", name=f"pt{t}_{rep}")
                            for t in range(2)
                        ]
                        for k in range(8):
                            nc.tensor.transpose(
                                out=pts[k // 4][:, 128 * (k % 4) : 128 * (k % 4) + 128],
                                in_=p1b[:, 16 * k : 16 * k + 16],
                                identity=cb[:, OB_IDT : OB_IDT + 128],
                            )
                        pTS = small.tile([16, 1024], BF16, name=f"pTS_{rep}")
                        for t in range(2):
                            nc.vector.tensor_copy(
                                out=pTS[:, 512 * t : 512 * t + 512], in_=pts[t][:]
                            )

                        pL2 = pqp.tile([128, 160], F32, name=f"pL2_{rep}")
                        for k in range(8):
                            nc.tensor.matmul(
                                pL2[:, 20 * k : 20 * k + 20],
                                lhsT=pTS[:, 128 * k : 128 * k + 128],
                                rhs=cb[0:16, OB_Q2 : OB_Q2 + 20],
                                start=True, stop=True,
                            )
                        E2 = small.tile([128, 160], BF16, name=f"E2_{rep}")
                        nc.scalar.activation(out=E2[:], in_=pL2[:], func=AF.Exp)

                        # Odd instances (partitions 64:128) are DMA-shifted to
                        # partition base 0: back-to-back matmuls whose operand
                        # base partition flips 0<->64 lock up the PE (probed).
                        E2lo = small.tile([64, 160], BF16, name=f"E2lo_{rep}")
                        nc.sync.dma_start(out=E2lo[:], in_=E2[64:128, :])
                        pv2lo = small.tile([64, 160], BF16, name=f"pv2lo_{rep}")
                        nc.sync.dma_start(out=pv2lo[:], in_=pv2[64:128, :])
                        pnd = pqp.tile([NCLS, 272], F32, name=f"pnd_{rep}")
                        for i in range(I):
                            k, hh = i // 2, i % 2
                            Esrc = E2 if hh == 0 else E2lo
                            vsrc = pv2 if hh == 0 else pv2lo
                            nc.tensor.matmul(
                                pnd[:, 17 * i : 17 * i + 17],
                                lhsT=Esrc[0:64, 20 * k : 20 * k + 19],
                                rhs=vsrc[0:64, 20 * k : 20 * k + 17],
                                start=True, stop=True,
                            )
                        pndS = small.tile([NCLS, 272], F32, name=f"pndS_{rep}")
                        nc.vector.tensor_copy(out=pndS[:], in_=pnd[:])
                        pndV = pndS[:].rearrange("p (i j) -> p i j", j=17)
                        recd2 = small.tile([NCLS, 16], F32, name=f"recd2_{rep}")
                        nc.vector.reciprocal(out=recd2[:], in_=pndV[:, :, 16:17])
                        z2t = small.tile([NCLS, 272], F32, name=f"z2t_{rep}")
                        nc.vector.tensor_mul(
                            out=z2t[:], in0=pndS[:], in1=cf[0:NCLS, OF_W2R : OF_W2R + 272]
                        )
                        s2 = small.tile([NCLS, 16], F32, name=f"s2_{rep}")
                        nc.vector.reduce_sum(
                            out=s2[:],
                            in_=z2t[:].rearrange("p (i j) -> p i j", j=17),
                            axis=mybir.AxisListType.X,
                        )
                        z2 = small.tile([NCLS, 16], F32, name=f"z2_{rep}")
                        nc.vector.tensor_mul(out=z2[:], in0=s2[:], in1=recd2[:])
                        ez2 = small.tile([NCLS, 16], F32, name=f"ez2_{rep}")
                        nc.scalar.activation(
                            out=ez2[:], in_=z2[:], func=AF.Exp, scale=-1.0,
                            bias=cf[0:NCLS, OF_NB2 : OF_NB2 + 1],
                        )
                        nc.vector.tensor_scalar_add(out=ez2[:], in0=ez2[:], scalar1=1.0)
                        nc.vector.reciprocal(out=outcls_sb[:], in_=ez2[:])

            nc.sync.dma_start(out=out_cls.rearrange("i c -> c i"), in_=outcls_sb[:])

    nc.compile()
    return nc


def _get_nc():
    if "nc" not in _CACHE:
        _CACHE["nc"] = _build_nc()
    return _CACHE["nc"]


def host_prep(inputs):
    """Build the per-core input maps (all numpy, host-side weight folding
    plus the point-gather of feat columns)."""
    f8 = np.float64
    w_pos = np.asarray(inputs["w_pos"], f8)          # (16, 18)
    W16 = w_pos[:, :16]
    w_d = w_pos[:, 16] - w_pos[:, 17]                # (16,)
    b_pos = np.asarray(inputs["b_pos"], f8)
    w_vote = np.asarray(inputs["w_vote"], f8)        # (8, 16, 32)
    b_vote = np.asarray(inputs["b_vote"], f8)        # (8, 16)
    Wp = np.asarray(inputs["w_poses"], f8).reshape(NCAPS, DCAP, CIN)
    b_poses = np.asarray(inputs["b_poses"], f8).reshape(NCAPS, DCAP)

    Weff = np.stack([W16 @ w_vote[n] @ Wp[n] for n in range(NCAPS)])  # (8,16,1280)
    beff = np.stack(
        [W16 @ (w_vote[n] @ b_poses[n] + b_vote[n]) + b_pos for n in range(NCAPS)]
    )
    Weff = Weff.reshape(128, CIN)
    beff = beff.reshape(128)
    wd_rep = np.tile(w_d, NCAPS)                     # (128,)

    Q1s = np.asarray(inputs["Q1"], f8) / 4.0         # (64, 16)
    BQ1 = np.zeros((128, 512), f8)
    for n in range(NCAPS):
        BQ1[n * 16 : (n + 1) * 16, n * 64 : (n + 1) * 64] = Q1s.T
    Wv1 = np.asarray(inputs["Wv1"], f8)
    BWV1 = np.zeros((128, 136), f8)
    for n in range(NCAPS):
        BWV1[n * 16 : (n + 1) * 16, n * 17 : n * 17 + 16] = Wv1
    EXP8REP = np.zeros((128, 136), f8)
    for m in range(4):
        for n in range(NCAPS):
            EXP8REP[32 * m + n, n * 17 : (n + 1) * 17] = 1.0

    # ---- cbf16 blob ----
    cbf16 = np.zeros((128, W_BF16), np.float64)
    weffT = Weff.T.reshape(KT, 128, 128).transpose(1, 0, 2).reshape(128, KT * 128)
    cbf16[:, OB_WEFF : OB_WEFF + KT * 128] = weffT
    waT = np.asarray(inputs["w_acts"], f8).T.reshape(KT, 128, 8)
    cbf16[:, OB_WA : OB_WA + KT * 8] = waT.transpose(1, 0, 2).reshape(128, KT * 8)
    cbf16[:, OB_BQ1 : OB_BQ1 + 512] = BQ1
    cbf16[:, OB_BWV : OB_BWV + 136] = BWV1
    cbf16[:, OB_E8 : OB_E8 + 136] = EXP8REP
    cbf16[0:16, OB_Q2 : OB_Q2 + NCLS] = (np.asarray(inputs["Q2"], f8) / 4.0).T
    cbf16[:, OB_IDT : OB_IDT + 128] = np.eye(128)
    cbf16[0:1, OB_ONE : OB_ONE + 8] = 1.0
    cbf16 = cbf16.astype(BF16_NP)

    # ---- cf32 blob (shared part; pegrid filled per image) ----
    cf32_base = np.zeros((128, W_F32), np.float32)
    wact1 = np.asarray(inputs["wact1"], np.float64)
    w1row = np.tile(np.concatenate([wact1, [0.0]]), NCAPS)        # (136,)
    cf32_base[:, OF_W1R : OF_W1R + 136] = w1row[None, :]
    wact2 = np.asarray(inputs["wact2"], np.float64)
    w2row = np.tile(np.concatenate([wact2, [0.0]]), I)            # (272,)
    cf32_base[0:NCLS, OF_W2R : OF_W2R + 272] = w2row[None, :]
    cf32_base[:, OF_NB1] = -float(np.asarray(inputs["bact1"]))
    cf32_base[0:NCLS, OF_NB2] = -float(np.asarray(inputs["bact2"]))
    bacts = np.asarray(inputs["b_acts"], np.float64)
    for m in range(4):
        cf32_base[32 * m : 32 * m + 8, OF_NBG] = -bacts

    feats = np.asarray(inputs["feature_output"])     # (8, 1280, 64, 64) f32
    coords = np.asarray(inputs["point_coords"])      # (8, 16, 2, 256) int32
    mask = np.asarray(inputs["point_mask"])          # (8, 16, 256) bool

    in_maps = []
    for b in range(B):
        y = np.clip(coords[b, :, 0, :], 0, HF - 1).astype(np.int64)
        x = np.clip(coords[b, :, 1, :], 0, WF - 1).astype(np.int64)
        sidx = (y * WF + x).reshape(NPTS)
        mb = mask[b].reshape(NPTS)

        fb = feats[b].reshape(CIN, S)
        feat_pts = np.empty((CIN + 1, NPTS), BF16_NP)
        feat_pts[0:CIN] = fb[:, sidx].astype(BF16_NP)
        feat_pts[CIN] = np.where(mb, 0.0, -30.0).astype(BF16_NP)

        yr = coords[b, :, 0, :].astype(np.float64).reshape(NPTS)
        xr = coords[b, :, 1, :].astype(np.float64).reshape(NPTS)
        r = (yr - xr) / 128.0
        cf32 = cf32_base.copy()
        cf32[:, OF_PEG : OF_PEG + NPTS] = (
            wd_rep[:, None] * r[None, :] + beff[:, None]
        ).astype(np.float32)

        in_maps.append(dict(feat=feat_pts, cf32=cf32, cbf16=cbf16))
    return in_maps


def kernel(**inputs):
    nc = _get_nc()
    in_maps = host_prep(inputs)
    res = bass_utils.run_bass_kernel_spmd(nc, in_maps, core_ids=list(range(B)))
    out = np.stack([np.asarray(res.results[b]["out_cls"]) for b in range(B)])
    return out.astype(np.float32)


# revision 28
# speedup vs baseline: 23.0746x; 1.0605x over previous
"""Trainium2 Bass kernel for nn_CapsuleModel2 (capsule routing head).

Strategy (data-parallel, one image per NeuronCore, 8 cores):

Host-side folding:
  1x1conv(poses) -> vote conv -> positional linear collapses into
     tokens[(n,v), j] = Weff @ feat_pts + pegrid_pts
  AND the point gather is hoisted to the host: feat columns arrive
  already permuted into point order (feat_pts[:, j] = feat[:, sidx[j]]),
  so the device computes tokens/acts DIRECTLY per point — no GPSIMD
  gather. Row 1280 of the feat input is a mask row (-30 for masked
  points) accumulated into the act logits via a 1-partition matmul.

Device pipeline per core (one image):
  G:  tokens[(n,v), j] (bf16, DVE add of psum+pegrid), act logits
      z[n, j] (+mask row), exp(-z-b) written straight into a
      4x32-partition-block quartered layout; 3 DVE ops finish
      sg = sigmoid+1e-6 as [128, 1024] bf16.
  R1: per 128-point chunk: L = tok.T @ blockdiag(Q1/4), E = exp(L) bf16;
      vals = sg * (tok.T @ blockdiag(Wv1)) bf16; numerator matmuls in
      the [o=64, j=17] orientation so all 16 instances accumulate into
      ONE psum bank [128, 17*8] and R2 runs fully batched.
  R2: batched across all 16 instances on [128,*]/[19,272] tiles:
      normalize, a1 sigmoid, Q2 attention, class sigmoid -> out [19,16].

Inputs are packed into 3 DRAM tensors (feat, cf32, cbf16) to minimize
per-dispatch argument overhead.

Hardware pitfall encoded here: back-to-back PE matmuls whose operand
base partition flips 0<->64 lock up the device (probed on HW), so the
R2 class-attention stage DMA-shifts odd instances to partition base 0.
"""

import sys

for _p in ("/opt/trn_rl_repo",):
    if _p not in sys.path:
        sys.path.insert(0, _p)

import numpy as np
import ml_dtypes

import concourse.bacc as bacc
import concourse.tile as tile
from concourse import mybir
from concourse import bass_utils

AF = mybir.ActivationFunctionType
ALU = mybir.AluOpType
F32 = mybir.dt.float32
BF16 = mybir.dt.bfloat16
BF16_NP = ml_dtypes.bfloat16

B, I, P = 8, 16, 256
CIN = 1280
NCAPS, DCAP, DV = 8, 32, 16
HF = WF = 64
S = HF * WF              # 4096 grid positions
NPTS = I * P             # 4096 points (== S by coincidence)
NOUT1, NCLS = 64, 19
KT = CIN // 128          # 10 contraction tiles
HALF = NPTS // 2
NCH = 32                 # routing-1 chunks of 128 points

# --- cbf16 blob column offsets (bf16) ---
OB_WEFF = 0                      # [128, KT*128] weffT
OB_WA = OB_WEFF + KT * 128       # [128, KT*8]   waT
OB_BQ1 = OB_WA + KT * 8          # [128, 512]    blockdiag Q1/4
OB_BWV = OB_BQ1 + 512            # [128, 136]    blockdiag Wv1 (col16=0)
OB_E8 = OB_BWV + 136             # [128, 136]    act-replication matrix
OB_Q2 = OB_E8 + 136              # [16, 20]      (Q2/4).T zero-padded
OB_IDT = OB_Q2 + 20              # [128, 128]    identity (transposes)
OB_ONE = OB_IDT + 128            # [1, 8]        ones (mask-row matmul)
W_BF16 = OB_ONE + 8              # 2300

# --- cf32 blob column offsets (f32; pegrid part is per-image) ---
OF_PEG = 0                       # [128, 4096] positional table at points
OF_W1R = OF_PEG + NPTS           # [128, 136]  wact1 tiled per block, col16=0
OF_W2R = OF_W1R + 136            # [19, 272]   wact2 tiled per block, col16=0
OF_NB1 = OF_W2R + 272            # [128, 1]    -bact1
OF_NB2 = OF_NB1 + 1              # [19, 1]     -bact2
OF_NBG = OF_NB2 + 1              # [128, 1]    row 32m+n = -bacts[n]
W_F32 = OF_NBG + 1               # 4507

_CACHE = {}


def _build_nc(repeat=1, phases=4):
    nc = bacc.Bacc("TRN2", target_bir_lowering=False, debug=False, num_devices=8)

    feat = nc.dram_tensor("feat", [CIN + 1, NPTS], BF16, kind="ExternalInput").ap()
    cf32 = nc.dram_tensor("cf32", [128, W_F32], F32, kind="ExternalInput").ap()
    cbf16 = nc.dram_tensor("cbf16", [128, W_BF16], BF16, kind="ExternalInput").ap()
    out_cls = nc.dram_tensor("out_cls", [I, NCLS], F32, kind="ExternalOutput").ap()

    with tile.TileContext(nc) as tc:
        with (
            tc.tile_pool(name="cons", bufs=1) as cons,
            tc.tile_pool(name="grid", bufs=1) as grid,
            tc.tile_pool(name="feats", bufs=4) as feats,
            tc.tile_pool(name="rsb", bufs=6) as rsb,
            tc.tile_pool(name="small", bufs=1) as small,
        ):
            # ---- constants via the gpsimd (SWDGE) queue; feat on sync ----
            cb = cons.tile([128, W_BF16], BF16)
            nc.gpsimd.dma_start(out=cb[:], in_=cbf16)
            cf = cons.tile([128, W_F32], F32)
            for jq in range(2):
                half = W_F32 // 2
                lo = jq * half
                hi = W_F32 if jq == 1 else half
                nc.gpsimd.dma_start(out=cf[:, lo:hi], in_=cf32[:, lo:hi])

            # ---- persistent tensors ----
            tokens_sb = grid.tile([128, NPTS], BF16)
            zq = grid.tile([128, NPTS // 4], F32)    # quartered act logits
            sg2 = grid.tile([128, NPTS // 4], BF16)  # sigmoid(z)+1e-6
            outcls_sb = grid.tile([NCLS, I], F32)
            nc.vector.memset(zq[:, :], 0.0)

            for rep in range(repeat):
                # ================= phase G: point matmuls ================
                with tc.tile_pool(name=f"pg{rep}", bufs=1, space="PSUM") as pg:
                    for h in range(2):
                        mrow = feats.tile([1, HALF], BF16, tag="mrow", name=f"mr{h}_{rep}")
                        nc.sync.dma_start(
                            out=mrow[:], in_=feat[CIN : CIN + 1, h * HALF : (h + 1) * HALF]
                        )
                        pms = [
                            pg.tile([128, 512], F32, tag=f"pm{nn}", name=f"pm{nn}_{h}_{rep}")
                            for nn in range(4)
                        ]
                        pas = [
                            pg.tile([8, 512], F32, tag=f"pa{nn}", name=f"pa{nn}_{h}_{rep}")
                            for nn in range(4)
                        ]
                        for k in range(KT):
                            ft = feats.tile([128, HALF], BF16, tag="feat")
                            nc.sync.dma_start(
                                out=ft[:],
                                in_=feat[k * 128 : (k + 1) * 128, h * HALF : (h + 1) * HALF],
                            )
                            for nn in range(4):
                                nc.tensor.matmul(
                                    pms[nn][:],
                                    lhsT=cb[:, OB_WEFF + k * 128 : OB_WEFF + (k + 1) * 128],
                                    rhs=ft[:, nn * 512 : (nn + 1) * 512],
                                    start=(k == 0),
                                    stop=(k == KT - 1),
                                )
                            for nn in range(4):
                                nc.tensor.matmul(
                                    pas[nn][:],
                                    lhsT=cb[:, OB_WA + k * 8 : OB_WA + (k + 1) * 8],
                                    rhs=ft[:, nn * 512 : (nn + 1) * 512],
                                    start=(k == 0),
                                    stop=False,
                                )
                        # mask row folds into the act logits
                        for nn in range(4):
                            nc.tensor.matmul(
                                pas[nn][:],
                                lhsT=cb[0:1, OB_ONE : OB_ONE + 8],
                                rhs=mrow[:, nn * 512 : (nn + 1) * 512],
                                start=False,
                                stop=True,
                            )
                        for nn in range(4):
                            off = h * HALF + nn * 512
                            q, loc = off // 1024, off % 1024
                            nc.vector.tensor_add(
                                out=tokens_sb[:, off : off + 512],
                                in0=pms[nn][:],
                                in1=cf[:, OF_PEG + off : OF_PEG + off + 512],
                            )
                            # exp(-z - bacts) straight into the quartered
                            # 32-row-block layout R1's broadcast matmul eats
                            nc.scalar.activation(
                                out=zq[32 * q : 32 * q + 8, loc : loc + 512],
                                in_=pas[nn][:],
                                func=AF.Exp,
                                scale=-1.0,
                                bias=cf[32 * q : 32 * q + 8, OF_NBG : OF_NBG + 1],
                            )

                if phases < 2:
                    nc.vector.memset(outcls_sb[:, :], 0.0)
                    continue

                # finish sg = 1/(1+exp(-z-b)) + 1e-6 on [128, 1024]
                nc.vector.tensor_scalar_add(out=zq[:], in0=zq[:], scalar1=1.0)
                nc.vector.reciprocal(out=zq[:], in_=zq[:])
                nc.vector.tensor_scalar_add(out=sg2[:], in0=zq[:], scalar1=1e-6)

                if phases < 3:
                    nc.vector.memset(outcls_sb[:, :], 0.0)
                    continue

                # ================= phase R1: routing 1 ===================
                with (
                    tc.tile_pool(name=f"pn{rep}", bufs=1, space="PSUM") as pnp,
                ):
                    pn_all = pnp.tile([128, 136], F32, name=f"pnall_{rep}")

                    with (
                        tc.tile_pool(name=f"pl{rep}", bufs=3, space="PSUM") as plp,
                        tc.tile_pool(name=f"pv{rep}", bufs=3, space="PSUM") as pvp,
                    ):
                        def emit_front(c):
                            tokc = tokens_sb[:, c * 128 : (c + 1) * 128]
                            m4 = c // 8
                            pl = plp.tile([128, 512], F32, tag="pl", name=f"pl{c}_{rep}")
                            nc.tensor.matmul(
                                pl[:], lhsT=tokc,
                                rhs=cb[:, OB_BQ1 : OB_BQ1 + 512],
                                start=True, stop=True,
                            )
                            E = rsb.tile([128, 512], BF16, tag="E", name=f"E{c}_{rep}")
                            nc.scalar.activation(out=E[:], in_=pl[:], func=AF.Exp)

                            pvpa = pvp.tile([128, 272], F32, tag="pv", name=f"pv{c}_{rep}")
                            nc.tensor.matmul(
                                pvpa[:, 0:136], lhsT=tokc,
                                rhs=cb[:, OB_BWV : OB_BWV + 136],
                                start=True, stop=True,
                            )
                            nc.tensor.matmul(
                                pvpa[:, 136:272],
                                lhsT=sg2[
                                    32 * m4 : 32 * m4 + 8,
                                    (c % 8) * 128 : (c % 8 + 1) * 128,
                                ],
                                rhs=cb[32 * m4 : 32 * m4 + 8, OB_E8 : OB_E8 + 136],
                                start=True, stop=True,
                                tile_position=(32 * m4, 0),
                            )
                            asig = rsb.tile([128, 136], BF16, tag="asig", name=f"as{c}_{rep}")
                            nc.vector.tensor_copy(out=asig[:], in_=pvpa[:, 136:272])
                            vals = rsb.tile([128, 136], BF16, tag="vals", name=f"va{c}_{rep}")
                            nc.vector.tensor_mul(
                                out=vals[:], in0=pvpa[:, 0:136], in1=asig[:]
                            )
                            vr = vals[:].rearrange("p (n j) -> p n j", j=17)
                            ar = asig[:].rearrange("p (n j) -> p n j", j=17)
                            nc.vector.tensor_copy(
                                out=vr[:, :, 16:17], in_=ar[:, :, 16:17]
                            )
                            return E, vals

                        def emit_numer(c, E, vals):
                            i = c // 2
                            po = 64 * (i % 2)
                            co = 17 * (i // 2)
                            for n in range(8):
                                nc.tensor.matmul(
                                    pn_all[po : po + 64, co : co + 17],
                                    lhsT=E[:, n * 64 : (n + 1) * 64],
                                    rhs=vals[:, n * 17 : (n + 1) * 17],
                                    start=(c % 2 == 0 and n == 0),
                                    stop=(c % 2 == 1 and n == 7),
                                    skip_group_check=True,
                                )

                        from collections import deque
                        pending = deque()
                        for c in range(NCH):
                            front = emit_front(c)
                            pending.append((c,) + front)
                            if len(pending) > 2:
                                emit_numer(*pending.popleft())
                        while pending:
                            emit_numer(*pending.popleft())

                    # ================= phase R2: batched =================
                    if phases < 4:
                        nc.vector.memset(outcls_sb[:, :], 0.0)
                        continue
                    with (
                        tc.tile_pool(name=f"pt{rep}", bufs=1, space="PSUM") as ptp,
                        tc.tile_pool(name=f"pq{rep}", bufs=1, space="PSUM") as pqp,
                    ):
                        pnS = small.tile([128, 136], F32, name=f"pnS_{rep}")
                        nc.vector.tensor_copy(out=pnS[:], in_=pn_all[:])
                        pnV = pnS[:].rearrange("p (i j) -> p i j", j=17)

                        recd = small.tile([128, 8], F32, name=f"recd_{rep}")
                        nc.vector.reciprocal(out=recd[:], in_=pnV[:, :, 16:17])

                        z1t = small.tile([128, 136], F32, name=f"z1t_{rep}")
                        nc.vector.tensor_mul(
                            out=z1t[:], in0=pnS[:], in1=cf[:, OF_W1R : OF_W1R + 136]
                        )
                        s1 = small.tile([128, 8], F32, name=f"s1_{rep}")
                        nc.vector.reduce_sum(
                            out=s1[:],
                            in_=z1t[:].rearrange("p (i j) -> p i j", j=17),
                            axis=mybir.AxisListType.X,
                        )
                        z1 = small.tile([128, 8], F32, name=f"z1_{rep}")
                        nc.vector.tensor_mul(out=z1[:], in0=s1[:], in1=recd[:])
                        a1e = small.tile([128, 8], F32, name=f"a1e_{rep}")
                        nc.scalar.activation(
                            out=a1e[:], in_=z1[:], func=AF.Exp, scale=-1.0,
                            bias=cf[:, OF_NB1 : OF_NB1 + 1],
                        )
                        nc.vector.tensor_scalar_add(out=a1e[:], in0=a1e[:], scalar1=1.0)
                        nc.vector.reciprocal(out=a1e[:], in_=a1e[:])
                        nc.vector.tensor_scalar_add(out=a1e[:], in0=a1e[:], scalar1=1e-6)
                        sc = small.tile([128, 8], F32, name=f"sc_{rep}")
                        nc.vector.tensor_mul(out=sc[:], in0=recd[:], in1=a1e[:])

                        p1b = small.tile([128, 128], BF16, name=f"p1b_{rep}")
                        # 20-col blocks keep pnd's rhs slices 4B-aligned
                        pv2 = small.tile([128, 160], BF16, name=f"pv2_{rep}")
                        nc.vector.memset(pv2[:, :], 0.0)
                        for k in range(8):
                            nc.vector.tensor_scalar_mul(
                                out=p1b[:, 16 * k : 16 * k + 16],
                                in0=pnS[:, 17 * k : 17 * k + 16],
                                scalar1=recd[:, k : k + 1],
                            )
                            nc.vector.tensor_scalar_mul(
                                out=pv2[:, 20 * k : 20 * k + 16],
                                in0=pnS[:, 17 * k : 17 * k + 16],
                                scalar1=sc[:, k : k + 1],
                            )
                        pv2V = pv2[:].rearrange("p (i j) -> p i j", j=20)
                        nc.vector.tensor_copy(out=pv2V[:, :, 16:17], in_=a1e[:])
                        # odd instances DMA-shifted to partition base 0 early
                        # (operand base flipping 0<->64 locks up the PE)
                        pv2lo = small.tile([64, 160], BF16, name=f"pv2lo_{rep}")
                        nc.sync.dma_start(out=pv2lo[:], in_=pv2[64:128, :])

                        # transposes: p1 [128,16] blocks -> [16,128] (2 inst)
                        pts = [
                            ptp.tile([16, 512], BF16, tag=f"pt{t}", name=f"pt{t}_{rep}")
                            for t in range(2)
                        ]
                        for k in range(8):
                            nc.tensor.transpose(
                                out=pts[k // 4][:, 128 * (k % 4) : 128 * (k % 4) + 128],
                                in_=p1b[:, 16 * k : 16 * k + 16],
                                identity=cb[:, OB_IDT : OB_IDT + 128],
                            )
                        pTS = small.tile([16, 1024], BF16, name=f"pTS_{rep}")
                        for t in range(2):
                            nc.vector.tensor_copy(
                                out=pTS[:, 512 * t : 512 * t + 512], in_=pts[t][:]
                            )

                        pL2 = pqp.tile([128, 160], F32, name=f"pL2_{rep}")
                        for k in range(8):
                            nc.tensor.matmul(
                                pL2[:, 20 * k : 20 * k + 20],
                                lhsT=pTS[:, 128 * k : 128 * k + 128],
                                rhs=cb[0:16, OB_Q2 : OB_Q2 + 20],
                                start=True, stop=True,
                            )
                        E2 = small.tile([128, 160], BF16, name=f"E2_{rep}")
                        nc.scalar.activation(out=E2[:], in_=pL2[:], func=AF.Exp)
                        E2lo = small.tile([64, 160], BF16, name=f"E2lo_{rep}")
                        nc.sync.dma_start(out=E2lo[:], in_=E2[64:128, :])
                        pnd = pqp.tile([NCLS, 272], F32, name=f"pnd_{rep}")
                        for i in range(I):
                            k, hh = i // 2, i % 2
                            Esrc = E2 if hh == 0 else E2lo
                            vsrc = pv2 if hh == 0 else pv2lo
                            nc.tensor.matmul(
                                pnd[:, 17 * i : 17 * i + 17],
                                lhsT=Esrc[0:64, 20 * k : 20 * k + 19],
                                rhs=vsrc[0:64, 20 * k : 20 * k + 17],
                                start=True, stop=True,
                            )
                        pndS = small.tile([NCLS, 272], F32, name=f"pndS_{rep}")
                        nc.vector.tensor_copy(out=pndS[:], in_=pnd[:])
                        pndV = pndS[:].rearrange("p (i j) -> p i j", j=17)
                        recd2 = small.tile([NCLS, 16], F32, name=f"recd2_{rep}")
                        nc.vector.reciprocal(out=recd2[:], in_=pndV[:, :, 16:17])
                        z2t = small.tile([NCLS, 272], F32, name=f"z2t_{rep}")
                        nc.vector.tensor_mul(
                            out=z2t[:], in0=pndS[:], in1=cf[0:NCLS, OF_W2R : OF_W2R + 272]
                        )
                        s2 = small.tile([NCLS, 16], F32, name=f"s2_{rep}")
                        nc.vector.reduce_sum(
                            out=s2[:],
                            in_=z2t[:].rearrange("p (i j) -> p i j", j=17),
                            axis=mybir.AxisListType.X,
                        )
                        z2 = small.tile([NCLS, 16], F32, name=f"z2_{rep}")
                        nc.vector.tensor_mul(out=z2[:], in0=s2[:], in1=recd2[:])
                        ez2 = small.tile([NCLS, 16], F32, name=f"ez2_{rep}")
                        nc.scalar.activation(
                            out=ez2[:], in_=z2[:], func=AF.Exp, scale=-1.0,
                            bias=cf[0:NCLS, OF_NB2 : OF_NB2 + 1],
                        )
                        nc.vector.tensor_scalar_add(out=ez2[:], in0=ez2[:], scalar1=1.0)
                        nc.vector.reciprocal(out=outcls_sb[:], in_=ez2[:])

            nc.sync.dma_start(out=out_cls.rearrange("i c -> c i"), in_=outcls_sb[:])

    nc.compile()
    return nc


def _get_nc():
    if "nc" not in _CACHE:
        _CACHE["nc"] = _build_nc()
    return _CACHE["nc"]


def host_prep(inputs):
    """Build the per-core input maps (all numpy, host-side weight folding
    plus the point-gather of feat columns)."""
    f8 = np.float64
    w_pos = np.asarray(inputs["w_pos"], f8)          # (16, 18)
    W16 = w_pos[:, :16]
    w_d = w_pos[:, 16] - w_pos[:, 17]                # (16,)
    b_pos = np.asarray(inputs["b_pos"], f8)
    w_vote = np.asarray(inputs["w_vote"], f8)        # (8, 16, 32)
    b_vote = np.asarray(inputs["b_vote"], f8)        # (8, 16)
    Wp = np.asarray(inputs["w_poses"], f8).reshape(NCAPS, DCAP, CIN)
    b_poses = np.asarray(inputs["b_poses"], f8).reshape(NCAPS, DCAP)

    Weff = np.stack([W16 @ w_vote[n] @ Wp[n] for n in range(NCAPS)])  # (8,16,1280)
    beff = np.stack(
        [W16 @ (w_vote[n] @ b_poses[n] + b_vote[n]) + b_pos for n in range(NCAPS)]
    )
    Weff = Weff.reshape(128, CIN)
    beff = beff.reshape(128)
    wd_rep = np.tile(w_d, NCAPS)                     # (128,)

    Q1s = np.asarray(inputs["Q1"], f8) / 4.0         # (64, 16)
    BQ1 = np.zeros((128, 512), f8)
    for n in range(NCAPS):
        BQ1[n * 16 : (n + 1) * 16, n * 64 : (n + 1) * 64] = Q1s.T
    Wv1 = np.asarray(inputs["Wv1"], f8)
    BWV1 = np.zeros((128, 136), f8)
    for n in range(NCAPS):
        BWV1[n * 16 : (n + 1) * 16, n * 17 : n * 17 + 16] = Wv1
    EXP8REP = np.zeros((128, 136), f8)
    for m in range(4):
        for n in range(NCAPS):
            EXP8REP[32 * m + n, n * 17 : (n + 1) * 17] = 1.0

    # ---- cbf16 blob ----
    cbf16 = np.zeros((128, W_BF16), np.float64)
    weffT = Weff.T.reshape(KT, 128, 128).transpose(1, 0, 2).reshape(128, KT * 128)
    cbf16[:, OB_WEFF : OB_WEFF + KT * 128] = weffT
    waT = np.asarray(inputs["w_acts"], f8).T.reshape(KT, 128, 8)
    cbf16[:, OB_WA : OB_WA + KT * 8] = waT.transpose(1, 0, 2).reshape(128, KT * 8)
    cbf16[:, OB_BQ1 : OB_BQ1 + 512] = BQ1
    cbf16[:, OB_BWV : OB_BWV + 136] = BWV1
    cbf16[:, OB_E8 : OB_E8 + 136] = EXP8REP
    cbf16[0:16, OB_Q2 : OB_Q2 + NCLS] = (np.asarray(inputs["Q2"], f8) / 4.0).T
    cbf16[:, OB_IDT : OB_IDT + 128] = np.eye(128)
    cbf16[0:1, OB_ONE : OB_ONE + 8] = 1.0
    cbf16 = cbf16.astype(BF16_NP)

    # ---- cf32 blob (shared part; pegrid filled per image) ----
    cf32_base = np.zeros((128, W_F32), np.float32)
    wact1 = np.asarray(inputs["wact1"], np.float64)
    w1row = np.tile(np.concatenate([wact1, [0.0]]), NCAPS)        # (136,)
    cf32_base[:, OF_W1R : OF_W1R + 136] = w1row[None, :]
    wact2 = np.asarray(inputs["wact2"], np.float64)
    w2row = np.tile(np.concatenate([wact2, [0.0]]), I)            # (272,)
    cf32_base[0:NCLS, OF_W2R : OF_W2R + 272] = w2row[None, :]
    cf32_base[:, OF_NB1] = -float(np.asarray(inputs["bact1"]))
    cf32_base[0:NCLS, OF_NB2] = -float(np.asarray(inputs["bact2"]))
    bacts = np.asarray(inputs["b_acts"], np.float64)
    for m in range(4):
        cf32_base[32 * m : 32 * m + 8, OF_NBG] = -bacts

    feats = np.asarray(inputs["feature_output"])     # (8, 1280, 64, 64) f32
    coords = np.asarray(inputs["point_coords"])      # (8, 16, 2, 256) int32
    mask = np.asarray(inputs["point_mask"])          # (8, 16, 256) bool

    in_maps = []
    for b in range(B):
        y = np.clip(coords[b, :, 0, :], 0, HF - 1).astype(np.int64)
        x = np.clip(coords[b, :, 1, :], 0, WF - 1).astype(np.int64)
        sidx = (y * WF + x).reshape(NPTS)
        mb = mask[b].reshape(NPTS)

        fb = feats[b].reshape(CIN, S)
        feat_pts = np.empty((CIN + 1, NPTS), BF16_NP)
        feat_pts[0:CIN] = fb[:, sidx].astype(BF16_NP)
        feat_pts[CIN] = np.where(mb, 0.0, -30.0).astype(BF16_NP)

        yr = coords[b, :, 0, :].astype(np.float64).reshape(NPTS)
        xr = coords[b, :, 1, :].astype(np.float64).reshape(NPTS)
        r = (yr - xr) / 128.0
        cf32 = cf32_base.copy()
        cf32[:, OF_PEG : OF_PEG + NPTS] = (
            wd_rep[:, None] * r[None, :] + beff[:, None]
        ).astype(np.float32)

        in_maps.append(dict(feat=feat_pts, cf32=cf32, cbf16=cbf16))
    return in_maps


def kernel(**inputs):
    nc = _get_nc()
    in_maps = host_prep(inputs)
    res = bass_utils.run_bass_kernel_spmd(nc, in_maps, core_ids=list(range(B)))
    out = np.stack([np.asarray(res.results[b]["out_cls"]) for b in range(B)])
    return out.astype(np.float32)
